# revision 1
# baseline (speedup 1.0000x reference)
"""Trainium2 Bass kernel for a pre-LN transformer block (B=4, T=2048, D=1024, H=16).

Sharding: 8 cores = (batch b = core//2) x (half p = core%2). Each core handles
1024 query tokens of its batch: p=0 -> 512-token blocks {0,3}, p=1 -> {1,2}
(balanced causal work). K/V are recomputed per core from the full batch
sequence (no collectives). Per-core variation (token selection, causal masks)
is carried entirely in input data so one uniform SPMD program serves all cores.

Layout: feature-major ("transposed") activations [D, tokens] so every matmul
uses weights as stored (lhsT = W chunk), attention scores/AV need no on-chip
transposes, and softmax denominators come from a ones-column appended to V.

Precision: all matmuls bf16 with fp32 PSUM accumulation; LN statistics,
softmax reciprocal, residuals in fp32. Row broadcasts (LN mu/rsig, softmax
reciprocal) are PE ones-matmuls with hi/lo bf16 splitting.

Scheduling: row-math (1-lane DVE) chains are software-pipelined one iteration
behind the matmul stream so the in-order PE queue never waits on them.
"""

import sys

sys.path.insert(0, "/opt/trn_rl_repo")

import numpy as np
import ml_dtypes

import concourse.bass as bass
import concourse.mybir as mybir
import concourse.tile as tile
from concourse import bacc
from concourse.bass_utils import run_bass_kernel_spmd

BF16 = mybir.dt.bfloat16
F32 = mybir.dt.float32
AF = mybir.ActivationFunctionType

B, T, D, H, HD = 4, 2048, 1024, 16, 64
EPS = 1e-5
P = 128
DC = D // P            # 8 feature chunks
S = 2                  # q slots per core (512 tokens each)
TCKV = T // 512        # 4 kv token 512-chunks
NKV = [8, 16]          # kv 128-blocks per slot (uniform across cores)
FC = 4 * D // P        # 32 ffn hidden chunks
NMASK = 16
QBLOCKS = [[0, 3], [1, 2]]

_built = {}


def _masked(s, j):
    return (s == 0) or (j >= 8)


def build_nc():
    nc = bacc.Bacc("TRN2", target_bir_lowering=False, debug=False, num_devices=8)

    d = {}
    d["xkv"] = nc.dram_tensor("xkv", [DC, TCKV, P, 512], BF16, kind="ExternalInput").ap()
    d["xqb"] = nc.dram_tensor("xqb", [DC, S, P, 512], BF16, kind="ExternalInput").ap()
    d["xqf"] = nc.dram_tensor("xqf", [DC, S, P, 512], F32, kind="ExternalInput").ap()
    d["wq"] = nc.dram_tensor("wq", [DC, P, DC, P], BF16, kind="ExternalInput").ap()
    d["wk"] = nc.dram_tensor("wk", [DC, P, DC, P], BF16, kind="ExternalInput").ap()
    d["wo"] = nc.dram_tensor("wo", [DC, P, DC, P], BF16, kind="ExternalInput").ap()
    d["wv"] = nc.dram_tensor("wv", [DC, P, D], BF16, kind="ExternalInput").ap()
    d["w1"] = nc.dram_tensor("w1", [FC, P, DC, P], BF16, kind="ExternalInput").ap()
    d["w2"] = nc.dram_tensor("w2", [DC, P, FC, P], BF16, kind="ExternalInput").ap()
    d["masks"] = nc.dram_tensor("masks", [NMASK, P, 512], BF16, kind="ExternalInput").ap()
    d["biasg"] = nc.dram_tensor("biasg", [P, 6 * DC], F32, kind="ExternalInput").ap()
    d["b1c"] = nc.dram_tensor("b1c", [P, FC], F32, kind="ExternalInput").ap()
    d["onesc"] = nc.dram_tensor("onesc", [P, P], BF16, kind="ExternalInput").ap()
    d["ident"] = nc.dram_tensor("ident", [64, 64], BF16, kind="ExternalInput").ap()
    d["epsv"] = nc.dram_tensor("epsv", [1, 1], F32, kind="ExternalInput").ap()
    d["outT"] = nc.dram_tensor("outT", [DC, S, P, 512], F32, kind="ExternalOutput").ap()

    with tile.TileContext(nc) as tc:
        _emit(nc, tc, d)
    nc.compile()
    return nc


def _emit(nc, tc, d):
    from contextlib import ExitStack

    with ExitStack() as es:
        consts = es.enter_context(tc.tile_pool(name="consts", bufs=1))

        ones = consts.tile([P, P], BF16, tag="ones", name="ones")
        nc.sync.dma_start(ones[:], d["onesc"][:])
        ident = consts.tile([64, 64], BF16, tag="ident", name="ident")
        nc.sync.dma_start(ident[:], d["ident"][:])
        biasg = consts.tile([P, 6 * DC], F32, tag="biasg", name="biasg")
        nc.sync.dma_start(biasg[:], d["biasg"][:])
        b1t = consts.tile([P, FC], F32, tag="b1t", name="b1t")
        nc.sync.dma_start(b1t[:], d["b1c"][:])
        epst = consts.tile([1, 1], F32, tag="epst", name="epst")
        nc.sync.dma_start(epst[:], d["epsv"][:])

        bo_col = lambda dc: biasg[:, dc:dc + 1]
        g1_col = lambda dc: biasg[:, DC + dc:DC + dc + 1]
        bl1_col = lambda dc: biasg[:, 2 * DC + dc:2 * DC + dc + 1]
        g2_col = lambda dc: biasg[:, 3 * DC + dc:3 * DC + dc + 1]
        bl2_col = lambda dc: biasg[:, 4 * DC + dc:4 * DC + dc + 1]
        b2_col = lambda dc: biasg[:, 5 * DC + dc:5 * DC + dc + 1]

        def layer_norm(chunks, g_col, b_col, pools, pfx):
            """chunks: list of (src_fn() -> xb tiles per dc, h_tiles per dc).
            Software-pipelined: chunk i's broadcast/apply is emitted after
            chunk i+1's stats so PE never waits on the DVE row chain."""
            ps_st, ps_bc, p_rows, p_tmp, p_sq = pools

            def stats(ci, src_fn):
                xbt = src_fn()
                s1 = ps_st.tile([1, 512], F32, tag="st", name=f"{pfx}s1_{ci}")
                s2 = ps_st.tile([1, 512], F32, tag="st", name=f"{pfx}s2_{ci}")
                sqs = []
                for dc in range(DC):
                    sq = p_sq.tile([P, 512], BF16, tag="sq", name=f"{pfx}sq_{dc}_{ci}")
                    nc.scalar.square(sq[:], xbt[dc][:])
                    sqs.append(sq)
                for dc in range(DC):
                    nc.tensor.matmul(s1[:], ones[:, 0:1], xbt[dc][:],
                                     start=(dc == 0), stop=(dc == DC - 1))
                for dc in range(DC):
                    nc.tensor.matmul(s2[:], ones[:, 0:1], sqs[dc][:],
                                     start=(dc == 0), stop=(dc == DC - 1))
                mu = p_rows.tile([1, 512], F32, tag="rows", name=f"{pfx}mu_{ci}")
                nc.vector.tensor_scalar_mul(mu[:], s1[:], 1.0 / D)
                msq = p_rows.tile([1, 512], F32, tag="rows", name=f"{pfx}ms_{ci}")
                nc.vector.tensor_scalar_mul(msq[:], s2[:], 1.0 / D)
                var = p_rows.tile([1, 512], F32, tag="rows", name=f"{pfx}va_{ci}")
                nc.vector.tensor_mul(var[:], mu[:], mu[:])
                nc.vector.tensor_sub(var[:], msq[:], var[:])
                sd = p_rows.tile([1, 512], F32, tag="rows", name=f"{pfx}sd_{ci}")
                nc.scalar.activation(sd[:], var[:], AF.Sqrt, bias=epst[:])
                rsig = p_rows.tile([1, 512], F32, tag="rows", name=f"{pfx}rs_{ci}")
                nc.vector.reciprocal(rsig[:], sd[:])
                cmu = p_rows.tile([1, 512], F32, tag="rows", name=f"{pfx}cm_{ci}")
                nc.vector.tensor_mul(cmu[:], mu[:], rsig[:])
                ah = p_rows.tile([1, 512], BF16, tag="rowsb", name=f"{pfx}ah_{ci}")
                nc.vector.tensor_copy(ah[:], rsig[:])
                al = p_rows.tile([1, 512], BF16, tag="rowsb", name=f"{pfx}al_{ci}")
                nc.vector.tensor_sub(al[:], rsig[:], ah[:])
                ch = p_rows.tile([1, 512], BF16, tag="rowsb", name=f"{pfx}ch_{ci}")
                nc.vector.tensor_copy(ch[:], cmu[:])
                cl = p_rows.tile([1, 512], BF16, tag="rowsb", name=f"{pfx}cl_{ci}")
                nc.vector.tensor_sub(cl[:], cmu[:], ch[:])
                return xbt, (ah, al, ch, cl)

            def apply(ci, xbt, rows, h_tiles):
                ah, al, ch, cl = rows
                bcA = ps_bc.tile([P, 512], F32, tag="bc", name=f"{pfx}bA_{ci}")
                nc.tensor.matmul(bcA[:], ones[0:1, :], ah[:], start=True, stop=False)
                nc.tensor.matmul(bcA[:], ones[0:1, :], al[:], start=False, stop=True)
                bcC = ps_bc.tile([P, 512], F32, tag="bc", name=f"{pfx}bC_{ci}")
                nc.tensor.matmul(bcC[:], ones[0:1, :], ch[:], start=True, stop=False)
                nc.tensor.matmul(bcC[:], ones[0:1, :], cl[:], start=False, stop=True)
                for dc in range(DC):
                    tmp = p_tmp.tile([P, 512], F32, tag="lntmp", name=f"{pfx}lt_{dc}_{ci}")
                    nc.vector.tensor_mul(tmp[:], xbt[dc][:], bcA[:])
                    nc.vector.tensor_sub(tmp[:], tmp[:], bcC[:])
                    nc.scalar.activation(h_tiles[dc][:], tmp[:], AF.Identity,
                                         bias=b_col(dc), scale=g_col(dc))

            pend = None
            for ci, (src_fn, h_tiles) in enumerate(chunks):
                xbt, rows = stats(ci, src_fn)
                if pend is not None:
                    apply(*pend)
                pend = (ci, xbt, rows, h_tiles)
            apply(*pend)

        # ---------- persistent pool: tags reused across disjoint lifetimes ----
        # pa0..31: h (P1-2) then ff1 (P5); pa32..47: hq (P1-2)
        # pb0..31: kt (P2-3) then h2 (pb0..15) / xb2 (pb16..31) (P4-5)
        # pv0..15: v (P2-3) then x2 (P4-5);  pc0..15: qt;  pt0..15: att
        p_main = es.enter_context(tc.tile_pool(name="p_main", bufs=1))

        h_t = [[p_main.tile([P, 512], BF16, tag=f"pa{dc * TCKV + t}", name=f"h_{dc}_{t}")
                for t in range(TCKV)] for dc in range(DC)]
        hq_t = [[p_main.tile([P, 512], BF16, tag=f"pa{32 + dc * S + s}", name=f"hq_{dc}_{s}")
                 for s in range(S)] for dc in range(DC)]

        # ---------- phases 1+2: LN1 and projections (shared scope) ----------
        with tc.tile_pool(name="p_xsrc", bufs=16) as p_xsrc, \
             tc.tile_pool(name="ps_st", bufs=2, space="PSUM") as ps_st, \
             tc.tile_pool(name="ps_bc", bufs=2, space="PSUM") as ps_bc, \
             tc.tile_pool(name="p_rows", bufs=4) as p_rows, \
             tc.tile_pool(name="p_tmp", bufs=4) as p_tmp, \
             tc.tile_pool(name="p_sq", bufs=8) as p_sq, \
             tc.tile_pool(name="p_wsl", bufs=2) as p_wsl, \
             tc.tile_pool(name="p_wvs", bufs=9) as p_wvs, \
             tc.tile_pool(name="ps_mm", bufs=4, space="PSUM") as ps_mm:

            def mk_src_kv(tcx):
                def f():
                    out = []
                    for dc in range(DC):
                        xt = p_xsrc.tile([P, 512], BF16, tag="xsrc", name=f"xkv_{dc}_{tcx}")
                        nc.sync.dma_start(xt[:], d["xkv"][dc, tcx])
                        out.append(xt)
                    return out
                return f

            def mk_src_q(s):
                def f():
                    out = []
                    for dc in range(DC):
                        xt = p_xsrc.tile([P, 512], BF16, tag="xsrc", name=f"xqb_{dc}_{s}")
                        nc.sync.dma_start(xt[:], d["xqb"][dc, s])
                        out.append(xt)
                    return out
                return f

            pools = (ps_st, ps_bc, p_rows, p_tmp, p_sq)
            chunks = [(mk_src_kv(t), [h_t[dc][t] for dc in range(DC)]) for t in range(TCKV)]
            chunks += [(mk_src_q(s), [hq_t[dc][s] for dc in range(DC)]) for s in range(S)]
            layer_norm(chunks, g1_col, bl1_col, pools, "a")

            # ------- projections KT, V, QT (overlap LN1) -------
            kt_t = [[p_main.tile([P, 512], BF16, tag=f"pb{dc * TCKV + t}", name=f"kt_{dc}_{t}")
                     for t in range(TCKV)] for dc in range(DC)]
            v_t = [p_main.tile([P, H * 65], BF16, tag=f"pv{j}", name=f"v_{j}")
                   for j in range(T // P)]
            qt_t = [[p_main.tile([P, 512], BF16, tag=f"pc{dc * S + s}", name=f"qt_{dc}_{s}")
                     for s in range(S)] for dc in range(DC)]
            for dc in range(DC):  # K^T
                wsl = p_wsl.tile([P, DC, P], BF16, tag="wk", name=f"wks_{dc}")
                nc.sync.dma_start(wsl[:], d["wk"][dc])
                for t in range(TCKV):
                    pt = ps_mm.tile([P, 512], F32, tag="mm", name=f"pk_{dc}_{t}")
                    for di in range(DC):
                        nc.tensor.matmul(pt[:], wsl[:, di], h_t[di][t][:],
                                         start=(di == 0), stop=(di == DC - 1))
                    nc.scalar.copy(kt_t[dc][t][:], pt[:])
            for j in range(T // P):  # ones columns of V
                nc.scalar.activation(
                    v_t[j].rearrange("p (h c) -> p h c", c=65)[:, :, 64:65],
                    ones[:, 0:H].unsqueeze(2), AF.Copy)
            for doc in range(2):  # V token-major
                wvs = [p_wvs.tile([P, 512], BF16, tag="wv", name=f"wvs_{di}_{doc}")
                       for di in range(DC)]
                for di in range(DC):
                    nc.sync.dma_start(wvs[di][:], d["wv"][di, :, doc * 512:(doc + 1) * 512])
                for j in range(T // P):
                    t5, jo = j // 4, (j % 4) * P
                    pt = ps_mm.tile([P, 512], F32, tag="mm", name=f"pv_{j}_{doc}")
                    for di in range(DC):
                        nc.tensor.matmul(pt[:], h_t[di][t5][:, jo:jo + P], wvs[di][:],
                                         start=(di == 0), stop=(di == DC - 1))
                    dst = v_t[j].rearrange("p (h c) -> p h c", c=65)[:, doc * 8:(doc + 1) * 8, 0:64]
                    nc.scalar.copy(dst, pt.rearrange("p (h c) -> p h c", c=64))
            for dc in range(DC):  # Q^T
                wsl = p_wsl.tile([P, DC, P], BF16, tag="wq", name=f"wqs_{dc}")
                nc.sync.dma_start(wsl[:], d["wq"][dc])
                for s in range(S):
                    pt = ps_mm.tile([P, 512], F32, tag="mm", name=f"pq_{dc}_{s}")
                    for di in range(DC):
                        nc.tensor.matmul(pt[:], wsl[:, di], hq_t[di][s][:],
                                         start=(di == 0), stop=(di == DC - 1))
                    nc.scalar.copy(qt_t[dc][s][:], pt[:])

        # ---------- phase 3: attention ----------
        att_t = [[p_main.tile([P, 512], BF16, tag=f"pt{dc * S + s}", name=f"at_{dc}_{s}")
                  for s in range(S)] for dc in range(DC)]

        with tc.tile_pool(name="ps_s", bufs=3, space="PSUM") as ps_s, \
             tc.tile_pool(name="ps_av", bufs=2, space="PSUM") as ps_av, \
             tc.tile_pool(name="ps_rb", bufs=2, space="PSUM") as ps_rb, \
             tc.tile_pool(name="ps_ap", bufs=1, space="PSUM") as ps_ap, \
             tc.tile_pool(name="p_es", bufs=6) as p_es, \
             tc.tile_pool(name="p_raw", bufs=5) as p_raw, \
             tc.tile_pool(name="p_rrow", bufs=2) as p_rrow, \
             tc.tile_pool(name="p_msk", bufs=1) as p_msk:
            maskt = []
            for mi in range(NMASK):
                mt = p_msk.tile([P, 512], BF16, tag=f"mask{mi}", name=f"mask{mi}")
                nc.sync.dma_start(mt[:], d["masks"][mi])
                maskt.append(mt)

            def div_emitter(s, hp, raws, rows):
                def emit():
                    for hh in range(2):
                        head = 2 * hp + hh
                        rec, rh, rl = rows[hh]
                        raw = raws[hh]
                        rb = ps_rb.tile([64, 512], F32, tag="rb", name=f"rb_{s}_{head}")
                        nc.tensor.matmul(rb[:], ones[64:65, 0:64], rh[64:65, :],
                                         start=True, stop=False)
                        nc.tensor.matmul(rb[:], ones[64:65, 0:64], rl[64:65, :],
                                         start=False, stop=True)
                        if hh == 0:
                            nc.vector.tensor_mul(att_t[hp][s][0:64, :], raw[:], rb[:])
                        else:
                            sc1 = p_raw.tile([64, 512], BF16, tag="scm", name=f"sm_{s}_{head}", bufs=2)
                            nc.vector.tensor_mul(sc1[:], raw[:], rb[:])
                            aps = ps_ap.tile([P, 512], F32, tag="ap", name=f"ap_{s}_{head}")
                            nc.tensor.matmul(aps[64:128, :], ident[:], sc1[:],
                                             start=True, stop=True)
                            nc.vector.tensor_copy(att_t[hp][s][64:128, :], aps[64:128, :])
                return emit

            pend = None
            for s in range(S):
                for hp in range(DC):
                    avp = [ps_av.tile([65, 512], F32, tag="av", name=f"av_{s}_{hp}_{hh}")
                           for hh in range(2)]
                    for j in range(NKV[s]):
                        for hh in range(2):
                            lo = hh * 64
                            sp = ps_s.tile([P, 512], F32, tag="sc", name=f"sc_{s}_{hp}_{j}_{hh}")
                            nc.tensor.matmul(
                                sp[:], kt_t[hp][j // 4][lo:lo + 64, (j % 4) * P:(j % 4) * P + P],
                                qt_t[hp][s][lo:lo + 64, :], start=True, stop=True)
                            es_ = p_es.tile([P, 512], BF16, tag="es", name=f"es_{s}_{hp}_{j}_{hh}")
                            nc.scalar.activation(es_[:], sp[:], AF.Exp, scale=HD ** -0.5)
                            if _masked(s, j):
                                nc.vector.tensor_mul(es_[:], es_[:], maskt[j][:])
                            nc.tensor.matmul(
                                avp[hh][:],
                                v_t[j].rearrange("p (h c) -> p h c", c=65)[:, 2 * hp + hh],
                                es_[:], start=(j == 0), stop=(j == NKV[s] - 1))
                    raws, rows = [], []
                    for hh in range(2):
                        head = 2 * hp + hh
                        raw = p_raw.tile([64, 512], BF16, tag="raw", name=f"rw_{s}_{head}")
                        nc.scalar.copy(raw[:], avp[hh][0:64, :])
                        den = p_rrow.tile([65, 512], F32, tag="den", name=f"dn_{s}_{head}", bufs=2)
                        nc.scalar.copy(den[64:65, :], avp[hh][64:65, :])
                        rec = p_rrow.tile([65, 512], F32, tag="rr", name=f"rc_{s}_{head}", bufs=2)
                        nc.vector.reciprocal(rec[64:65, :], den[64:65, :])
                        rh = p_rrow.tile([65, 512], BF16, tag="rrb", name=f"rh_{s}_{head}", bufs=8)
                        nc.vector.tensor_copy(rh[64:65, :], rec[64:65, :])
                        rl = p_rrow.tile([65, 512], BF16, tag="rrb", name=f"rl_{s}_{head}", bufs=8)
                        nc.vector.tensor_sub(rl[64:65, :], rec[64:65, :], rh[64:65, :])
                        raws.append(raw)
                        rows.append((rec, rh, rl))
                    if pend is not None:
                        pend()
                    pend = div_emitter(s, hp, raws, rows)
            pend()

        # ---------- phase 4: O-projection + residual + LN2 ----------
        x2_t = [[p_main.tile([P, 512], F32, tag=f"pv{dc * S + s}", name=f"x2_{dc}_{s}")
                 for s in range(S)] for dc in range(DC)]
        h2_t = [[p_main.tile([P, 512], BF16, tag=f"pb{dc * S + s}", name=f"h2_{dc}_{s}")
                 for s in range(S)] for dc in range(DC)]

        with tc.tile_pool(name="p_wsl2", bufs=2) as p_wsl2, \
             tc.tile_pool(name="p_xqf", bufs=3) as p_xqf, \
             tc.tile_pool(name="p_otmp", bufs=4) as p_otmp, \
             tc.tile_pool(name="ps_mm2", bufs=4, space="PSUM") as ps_mm2, \
             tc.tile_pool(name="ps_st2", bufs=2, space="PSUM") as ps_st2, \
             tc.tile_pool(name="ps_bc2", bufs=2, space="PSUM") as ps_bc2, \
             tc.tile_pool(name="p_rows2", bufs=3) as p_rows2, \
             tc.tile_pool(name="p_tmp2", bufs=4) as p_tmp2, \
             tc.tile_pool(name="p_sq2", bufs=8) as p_sq2:
            xb2_t = [[p_main.tile([P, 512], BF16, tag=f"pb{16 + dc * S + s}", name=f"xb2_{dc}_{s}")
                      for s in range(S)] for dc in range(DC)]
            for dc in range(DC):
                wsl = p_wsl2.tile([P, DC, P], BF16, tag="wo", name=f"wos_{dc}")
                nc.sync.dma_start(wsl[:], d["wo"][dc])
                for s in range(S):
                    pt = ps_mm2.tile([P, 512], F32, tag="mm2", name=f"po_{dc}_{s}")
                    for di in range(DC):
                        nc.tensor.matmul(pt[:], wsl[:, di], att_t[di][s][:],
                                         start=(di == 0), stop=(di == DC - 1))
                    ot = p_otmp.tile([P, 512], F32, tag="ot", name=f"o_{dc}_{s}")
                    nc.scalar.activation(ot[:], pt[:], AF.Identity, bias=bo_col(dc))
                    xqf = p_xqf.tile([P, 512], F32, tag="xqf", name=f"xqf_{dc}_{s}")
                    nc.sync.dma_start(xqf[:], d["xqf"][dc, s])
                    nc.vector.tensor_add(x2_t[dc][s][:], xqf[:], ot[:])
                    nc.scalar.copy(xb2_t[dc][s][:], x2_t[dc][s][:])
            pools2 = (ps_st2, ps_bc2, p_rows2, p_tmp2, p_sq2)
            chunks2 = [(lambda s=s: [xb2_t[dc][s] for dc in range(DC)],
                        [h2_t[dc][s] for dc in range(DC)]) for s in range(S)]
            layer_norm(chunks2, g2_col, bl2_col, pools2, "c")

        # ---------- phase 5: FFN + residual + output ----------
        with tc.tile_pool(name="p_w1s", bufs=3) as p_w1s, \
             tc.tile_pool(name="p_w2s", bufs=2) as p_w2s, \
             tc.tile_pool(name="p_fout", bufs=4) as p_fout, \
             tc.tile_pool(name="p_out", bufs=4) as p_out, \
             tc.tile_pool(name="ps_mm3", bufs=6, space="PSUM") as ps_mm3:
            ff1_t = [p_main.tile([P, 512], BF16, tag=f"pa{fc}", name=f"ff1_{fc}")
                     for fc in range(FC)]
            for s in range(S):
                for fc in range(FC):
                    w1s = p_w1s.tile([P, DC, P], BF16, tag="w1s", name=f"w1s_{s}_{fc}")
                    nc.sync.dma_start(w1s[:], d["w1"][fc])
                    pt = ps_mm3.tile([P, 512], F32, tag="mm3", name=f"pf_{fc}_{s}")
                    for di in range(DC):
                        nc.tensor.matmul(pt[:], w1s[:, di], h2_t[di][s][:],
                                         start=(di == 0), stop=(di == DC - 1))
                    nc.scalar.activation(ff1_t[fc][:], pt[:], AF.Relu,
                                         bias=b1t[:, fc:fc + 1])
                for dc in range(DC):
                    w2s = p_w2s.tile([P, FC, P], BF16, tag="w2s", name=f"w2s_{s}_{dc}")
                    nc.sync.dma_start(w2s[:], d["w2"][dc])
                    pt = ps_mm3.tile([P, 512], F32, tag="mm3", name=f"pg_{dc}_{s}")
                    for fc in range(FC):
                        nc.tensor.matmul(pt[:], w2s[:, fc], ff1_t[fc][:],
                                         start=(fc == 0), stop=(fc == FC - 1))
                    f2 = p_fout.tile([P, 512], F32, tag="f2", name=f"f2_{dc}_{s}")
                    nc.scalar.activation(f2[:], pt[:], AF.Relu, bias=b2_col(dc))
                    ou = p_out.tile([P, 512], F32, tag="ou", name=f"ou_{dc}_{s}")
                    nc.vector.tensor_add(ou[:], x2_t[dc][s][:], f2[:])
                    nc.sync.dma_start(d["outT"][dc, s], ou[:])


# ============================ host side ============================

def _slab(w, rows_chunks, cols_chunks):
    r, c = w.shape
    return np.ascontiguousarray(
        w.reshape(rows_chunks, r // rows_chunks, cols_chunks, c // cols_chunks)
        .transpose(2, 1, 0, 3)).astype(ml_dtypes.bfloat16)


def _prep_core(inputs, core):
    b, p = core // 2, core % 2
    bf16 = ml_dtypes.bfloat16
    x = np.asarray(inputs["x"], np.float32)[b]
    xT = np.ascontiguousarray(x.T)
    qb = QBLOCKS[p]
    qidx = np.concatenate([np.arange(q_ * 512, q_ * 512 + 512) for q_ in qb])
    xqT = np.ascontiguousarray(xT[:, qidx])

    m = {}
    m["xkv"] = np.ascontiguousarray(
        xT.reshape(DC, P, TCKV, 512).transpose(0, 2, 1, 3)).astype(bf16)
    xq4 = np.ascontiguousarray(xqT.reshape(DC, P, S, 512).transpose(0, 2, 1, 3))
    m["xqb"] = xq4.astype(bf16)
    m["xqf"] = xq4.astype(np.float32)
    m["wq"] = _slab(np.asarray(inputs["Wq"], np.float32), DC, DC)
    m["wk"] = _slab(np.asarray(inputs["Wk"], np.float32), DC, DC)
    m["wo"] = _slab(np.asarray(inputs["Wo"], np.float32), DC, DC)
    m["wv"] = np.ascontiguousarray(
        np.asarray(inputs["Wv"], np.float32).reshape(DC, P, D)).astype(bf16)
    m["w1"] = _slab(np.asarray(inputs["W1"], np.float32), DC, FC)
    m["w2"] = _slab(np.asarray(inputs["W2"], np.float32), FC, DC)

    masks = np.zeros((NMASK, P, 512), np.float32)
    for s in range(S):
        qstart = qb[s] * 512
        for j in (range(8) if s == 0 else range(8, 16)):
            kv = j * P + np.arange(P)[:, None]
            qg = qstart + np.arange(512)[None, :]
            masks[j] = (kv <= qg).astype(np.float32)
    m["masks"] = masks.astype(bf16)

    biasg = np.zeros((P, 6 * DC), np.float32)
    for i, key in enumerate(["bo", "ln1_g", "ln1_b", "ln2_g", "ln2_b", "b2"]):
        biasg[:, i * DC:(i + 1) * DC] = np.asarray(inputs[key], np.float32).reshape(DC, P).T
    m["biasg"] = np.ascontiguousarray(biasg)
    m["b1c"] = np.ascontiguousarray(
        np.asarray(inputs["b1"], np.float32).reshape(FC, P).T)
    m["onesc"] = np.ones((P, P), bf16)
    m["ident"] = np.eye(64, dtype=np.float32).astype(bf16)
    m["epsv"] = np.full((1, 1), EPS, np.float32)
    return m


def kernel(**inputs):
    if "nc" not in _built:
        _built["nc"] = build_nc()
    nc = _built["nc"]
    in_maps = [_prep_core(inputs, c) for c in range(8)]
    res = run_bass_kernel_spmd(nc, in_maps, core_ids=list(range(8)))
    out = np.zeros((B, T, D), np.float32)
    for c in range(8):
        b, p = c // 2, c % 2
        o = np.asarray(res.results[c]["outT"])
        for s in range(S):
            qb = QBLOCKS[p][s]
            blk = o[:, s].reshape(D, 512)
            out[b, qb * 512:(qb + 1) * 512, :] = blk.T
    return out.astype(np.float32)



# revision 43
# speedup vs baseline: 1.1001x; 1.1001x over previous
"""Trainium2 Bass kernel for a pre-LN transformer block (B=4, T=2048, D=1024, H=16).

Sharding: 8 cores = (batch b = core//2) x (half p = core%2). Each core handles
1024 query tokens of its batch: p=0 -> 512-token blocks {0,3}, p=1 -> {1,2}
(balanced causal work). K/V are recomputed per core from the full batch
sequence (no collectives). Per-core variation (token selection, causal masks)
is carried entirely in input data so one uniform SPMD program serves all cores.

Layout: feature-major ("transposed") activations [D, tokens] so every matmul
uses weights as stored (lhsT = W chunk), attention scores/AV need no on-chip
transposes, and softmax denominators come from a ones-column appended to V.

v2 scheduling: software-pipelined so the PE never idles long enough for the
HAM clock gate to re-throttle it:
  P1: LN1 + Q(slot0) + K(t0,t1)
  P2: attention slot0 (8 kv blocks) + fillers: V(all 16), K(t2,t3), Q(slot1)
  P3: attention slot1 (16 kv blocks) + fillers: O/LN2/FFN of slot0
  P4: O/LN2/FFN of slot1
Divisions run one head-pair behind their attention loop. Softmax reciprocals
use reciprocal_approx_fast in-place in PSUM; LN uses a single ACT Rsqrt; x2
residuals round-trip through DRAM scratch to stay under the SBUF budget.
"""

import sys

sys.path.insert(0, "/opt/trn_rl_repo")

import numpy as np
import ml_dtypes

import concourse.bass as bass
import concourse.mybir as mybir
import concourse.tile as tile
from concourse import bacc
from concourse.bass_utils import run_bass_kernel_spmd

BF16 = mybir.dt.bfloat16
F32 = mybir.dt.float32
AF = mybir.ActivationFunctionType

B, T, D, H, HD = 4, 2048, 1024, 16, 64
EPS = 1e-5
P = 128
DC = D // P            # 8 feature chunks
S = 2                  # q slots per core (512 tokens each)
TCKV = T // 512        # 4 kv token 512-chunks
NKV = [8, 16]          # kv 128-blocks per slot (uniform across cores)
FC = 4 * D // P        # 32 ffn hidden chunks
NMASK = 16
QBLOCKS = [[0, 3], [1, 2]]

_built = {}
DBG = False  # when True, adds a "dbg" output tensor with intermediate dumps


def _masked(s, j):
    return (s == 0) or (j >= 8)


def build_nc():
    nc = bacc.Bacc("TRN2", target_bir_lowering=False, debug=False, num_devices=8)

    d = {}
    d["xkv"] = nc.dram_tensor("xkv", [DC, TCKV, P, 512], BF16, kind="ExternalInput").ap()
    d["xqb"] = nc.dram_tensor("xqb", [DC, S, P, 512], BF16, kind="ExternalInput").ap()
    d["xqf"] = nc.dram_tensor("xqf", [DC, S, P, 512], F32, kind="ExternalInput").ap()
    d["wq"] = nc.dram_tensor("wq", [DC, P, DC, P], BF16, kind="ExternalInput").ap()
    d["wk"] = nc.dram_tensor("wk", [DC, P, DC, P], BF16, kind="ExternalInput").ap()
    d["wo"] = nc.dram_tensor("wo", [DC, P, DC, P], BF16, kind="ExternalInput").ap()
    d["wv"] = nc.dram_tensor("wv", [DC, P, D], BF16, kind="ExternalInput").ap()
    d["w1"] = nc.dram_tensor("w1", [FC, P, DC, P], BF16, kind="ExternalInput").ap()
    d["w2"] = nc.dram_tensor("w2", [DC, P, FC, P], BF16, kind="ExternalInput").ap()
    d["masks"] = nc.dram_tensor("masks", [NMASK, P, 512], BF16, kind="ExternalInput").ap()
    d["biasg"] = nc.dram_tensor("biasg", [P, 6 * DC], F32, kind="ExternalInput").ap()
    d["b1c"] = nc.dram_tensor("b1c", [P, FC], F32, kind="ExternalInput").ap()
    d["onesc"] = nc.dram_tensor("onesc", [P, P], BF16, kind="ExternalInput").ap()
    d["ident"] = nc.dram_tensor("ident", [64, 64], BF16, kind="ExternalInput").ap()
    d["epsv"] = nc.dram_tensor("epsv", [1, 1], F32, kind="ExternalInput").ap()
    if DBG:
        d["dbg"] = nc.dram_tensor("dbg", [20, P, 512], BF16, kind="ExternalOutput").ap()
    d["outT"] = nc.dram_tensor("outT", [DC, S, P, 512], F32, kind="ExternalOutput").ap()

    with tile.TileContext(nc) as tc:
        _emit(nc, tc, d)
    nc.compile()
    return nc


def _emit(nc, tc, d):
    from contextlib import ExitStack

    with ExitStack() as es:
        consts = es.enter_context(tc.tile_pool(name="consts", bufs=1))

        ones = consts.tile([P, P], BF16, tag="ones", name="ones")
        nc.sync.dma_start(ones[:], d["onesc"][:])
        ident = consts.tile([64, 64], BF16, tag="ident", name="ident")
        nc.sync.dma_start(ident[:], d["ident"][:])
        biasg = consts.tile([P, 6 * DC], F32, tag="biasg", name="biasg")
        nc.sync.dma_start(biasg[:], d["biasg"][:])
        b1t = consts.tile([P, FC], F32, tag="b1t", name="b1t")
        nc.sync.dma_start(b1t[:], d["b1c"][:])
        epst = consts.tile([1, 1], F32, tag="epst", name="epst")
        nc.sync.dma_start(epst[:], d["epsv"][:])

        bo_col = lambda dc: biasg[:, dc:dc + 1]
        g1_col = lambda dc: biasg[:, DC + dc:DC + dc + 1]
        bl1_col = lambda dc: biasg[:, 2 * DC + dc:2 * DC + dc + 1]
        g2_col = lambda dc: biasg[:, 3 * DC + dc:3 * DC + dc + 1]
        bl2_col = lambda dc: biasg[:, 4 * DC + dc:4 * DC + dc + 1]
        b2_col = lambda dc: biasg[:, 5 * DC + dc:5 * DC + dc + 1]

        # ---------- persistent tiles (tags reused across disjoint lifetimes) --
        # pa0..31: h (P1-P2) then ff1 (P3: s0, P4: s1)
        # pa32..47: hq (P1-P2) then xb2/h2 of s0 (P3+)
        # pb0..31: kt (P1-P3); pb{4dc},pb{4dc+1} then xb2/h2 of s1 (P4)
        # pv0..15: v;  pc0..15: qt;  pt0..15: att
        p_main = es.enter_context(tc.tile_pool(name="p_main", bufs=1))

        h_t = [[p_main.tile([P, 512], BF16, tag=f"pa{dc * TCKV + t}", name=f"h_{dc}_{t}")
                for t in range(TCKV)] for dc in range(DC)]
        hq_t = [[p_main.tile([P, 512], BF16, tag=f"pa{32 + dc * S + s}", name=f"hq_{dc}_{s}")
                 for s in range(S)] for dc in range(DC)]
        kt_t = [[p_main.tile([P, 512], BF16, tag=f"pb{dc * TCKV + t}", name=f"kt_{dc}_{t}")
                 for t in range(TCKV)] for dc in range(DC)]
        v_t = [p_main.tile([P, H * 65], BF16, tag=f"pv{j}", name=f"v_{j}")
               for j in range(T // P)]
        qt_t = [[p_main.tile([P, 512], BF16, tag=f"pc{dc * S + s}", name=f"qt_{dc}_{s}")
                 for s in range(S)] for dc in range(DC)]
        att_t = [[p_main.tile([P, 512], BF16, tag=f"pt{dc * S + s}", name=f"at_{dc}_{s}")
                  for s in range(S)] for dc in range(DC)]

        # =================== P1: LN1 + Q(s0) + K(t0,t1) =====================
        with tc.tile_pool(name="p_xsrc", bufs=16) as p_xsrc, \
             tc.tile_pool(name="p_sq", bufs=8) as p_sq, \
             tc.tile_pool(name="p_rows", bufs=4) as p_rows, \
             tc.tile_pool(name="p_tmp", bufs=4) as p_tmp, \
             tc.tile_pool(name="p_wsl", bufs=2) as p_wsl, \
             tc.tile_pool(name="ps_st", bufs=2, space="PSUM") as ps_st, \
             tc.tile_pool(name="ps_bc", bufs=2, space="PSUM") as ps_bc, \
             tc.tile_pool(name="ps_mm", bufs=4, space="PSUM") as ps_mm:

            def ln_stats(ci, src_fn, pfx):
                xbt = src_fn()
                st1 = ps_st.tile([P, 512], F32, tag="st", name=f"{pfx}s1_{ci}")
                st2 = ps_st.tile([P, 512], F32, tag="st", name=f"{pfx}s2_{ci}")
                sqs = []
                for dc in range(DC):
                    sq = p_sq.tile([P, 512], BF16, tag="sq", name=f"{pfx}sq_{dc}_{ci}")
                    nc.vector.tensor_mul(sq[:], xbt[dc][:], xbt[dc][:])
                    sqs.append(sq)
                for dc in range(DC):
                    nc.tensor.matmul(st1[0:1, :], ones[:, 0:1], xbt[dc][:],
                                     start=(dc == 0), stop=(dc == DC - 1))
                for dc in range(DC):
                    nc.tensor.matmul(st2[0:1, :], ones[:, 0:1], sqs[dc][:],
                                     start=(dc == 0), stop=(dc == DC - 1))
                mu = p_rows.tile([1, 512], F32, tag="rf", name=f"{pfx}mu_{ci}")
                nc.vector.tensor_scalar_mul(mu[:], st1[0:1, :], 1.0 / D)
                msq = p_rows.tile([1, 512], F32, tag="rf", name=f"{pfx}ms_{ci}")
                nc.vector.tensor_scalar_mul(msq[:], st2[0:1, :], 1.0 / D)
                var = p_rows.tile([1, 512], F32, tag="rf", name=f"{pfx}va_{ci}")
                nc.vector.tensor_mul(var[:], mu[:], mu[:])
                nc.vector.tensor_sub(var[:], msq[:], var[:])
                sd = p_rows.tile([1, 512], F32, tag="rf", name=f"{pfx}sd_{ci}")
                nc.scalar.activation(sd[:], var[:], AF.Sqrt, bias=epst[:])
                rsig = p_rows.tile([1, 512], F32, tag="rf", name=f"{pfx}rs_{ci}")
                nc.vector.reciprocal_approx_fast(out=rsig[:], in_=sd[:])
                cmu = p_rows.tile([1, 512], F32, tag="rf", name=f"{pfx}cm_{ci}")
                nc.vector.tensor_mul(cmu[:], mu[:], rsig[:])
                ah = p_rows.tile([1, 512], BF16, tag="rb", name=f"{pfx}ah_{ci}")
                nc.vector.tensor_copy(ah[:], rsig[:])
                ch = p_rows.tile([1, 512], BF16, tag="rb", name=f"{pfx}ch_{ci}")
                nc.vector.tensor_copy(ch[:], cmu[:])
                return xbt, (ah, ch)

            def ln_apply(ci, xbt, rows, h_tiles, pfx):
                ah, ch = rows
                bcA = ps_bc.tile([P, 512], F32, tag="bc", name=f"{pfx}bA_{ci}")
                nc.tensor.matmul(bcA[:], ones[0:1, :], ah[:], start=True, stop=True)
                bcC = ps_bc.tile([P, 512], F32, tag="bc", name=f"{pfx}bC_{ci}")
                nc.tensor.matmul(bcC[:], ones[0:1, :], ch[:], start=True, stop=True)
                for dc in range(DC):
                    tmp = p_tmp.tile([P, 512], F32, tag="lntmp", name=f"{pfx}lt_{dc}_{ci}")
                    nc.vector.tensor_mul(tmp[:], xbt[dc][:], bcA[:])
                    nc.vector.tensor_sub(tmp[:], tmp[:], bcC[:])
                    nc.scalar.activation(h_tiles[dc][:], tmp[:], AF.Identity,
                                         bias=bl1_col(dc), scale=g1_col(dc))

            def mk_src_kv(tcx):
                def f():
                    out = []
                    for dc in range(DC):
                        xt = p_xsrc.tile([P, 512], BF16, tag="xsrc", name=f"xkv_{dc}_{tcx}")
                        nc.sync.dma_start(xt[:], d["xkv"][dc, tcx])
                        out.append(xt)
                    return out
                return f

            def mk_src_q(s):
                def f():
                    out = []
                    for dc in range(DC):
                        xt = p_xsrc.tile([P, 512], BF16, tag="xsrc", name=f"xqb_{dc}_{s}")
                        nc.sync.dma_start(xt[:], d["xqb"][dc, s])
                        out.append(xt)
                    return out
                return f

            def q_proj1():
                for dc in range(DC):
                    wsl = p_wsl.tile([P, DC, P], BF16, tag="wsl", name=f"wqs_{dc}")
                    nc.sync.dma_start(wsl[:], d["wq"][dc])
                    pt = ps_mm.tile([P, 512], F32, tag="mm", name=f"pq_{dc}_0")
                    for di in range(DC):
                        nc.tensor.matmul(pt[:], wsl[:, di], hq_t[di][0][:],
                                         start=(di == 0), stop=(di == DC - 1))
                    nc.vector.tensor_copy(qt_t[dc][0][:], pt[:])

            def k_proj1():
                for dc in range(DC):
                    wsl = p_wsl.tile([P, DC, P], BF16, tag="wsl", name=f"wks_{dc}")
                    nc.sync.dma_start(wsl[:], d["wk"][dc])
                    for t in range(2):
                        pt = ps_mm.tile([P, 512], F32, tag="mm", name=f"pk_{dc}_{t}")
                        for di in range(DC):
                            nc.tensor.matmul(pt[:], wsl[:, di], h_t[di][t][:],
                                             start=(di == 0), stop=(di == DC - 1))
                        nc.vector.tensor_copy(kt_t[dc][t][:], pt[:])

            # chunk order: qS, t0, t1, qB, t2, t3; apply lags one chunk behind
            chunks = [(mk_src_q(0), [hq_t[dc][0] for dc in range(DC)], "aq0"),
                      (mk_src_kv(0), [h_t[dc][0] for dc in range(DC)], "at0"),
                      (mk_src_kv(1), [h_t[dc][1] for dc in range(DC)], "at1"),
                      (mk_src_q(1), [hq_t[dc][1] for dc in range(DC)], "aq1"),
                      (mk_src_kv(2), [h_t[dc][2] for dc in range(DC)], "at2"),
                      (mk_src_kv(3), [h_t[dc][3] for dc in range(DC)], "at3")]
            pend = None
            post = {1: q_proj1, 3: k_proj1}
            for ci, (src_fn, h_tiles, pfx) in enumerate(chunks):
                xbt, rows = ln_stats(ci, src_fn, pfx)
                if pend is not None:
                    ln_apply(pend[0], pend[1], pend[2], pend[3], pend[4])
                pend = (ci, xbt, rows, h_tiles, pfx)
                if ci in post:
                    post[ci]()
            ln_apply(pend[0], pend[1], pend[2], pend[3], pend[4])
            if DBG:
                nc.sync.dma_start(d["dbg"][0], h_t[0][0][:])
                nc.sync.dma_start(d["dbg"][2], hq_t[0][0][:])
                nc.sync.dma_start(d["dbg"][3], hq_t[0][1][:])
                nc.sync.dma_start(d["dbg"][4], qt_t[0][0][:])
                nc.sync.dma_start(d["dbg"][5], kt_t[0][0][:])

        # =================== attention-wide scope (P2..P4) ==================
        with tc.tile_pool(name="p_es", bufs=3) as p_es, \
             tc.tile_pool(name="p_raw", bufs=4) as p_raw, \
             tc.tile_pool(name="p_rhb", bufs=4) as p_rhb, \
             tc.tile_pool(name="p_rec", bufs=1) as p_rec, \
             tc.tile_pool(name="p_sc1", bufs=1) as p_sc1, \
             tc.tile_pool(name="ps_sc", bufs=2, space="PSUM") as ps_sc, \
             tc.tile_pool(name="ps_av", bufs=2, space="PSUM") as ps_av, \
             tc.tile_pool(name="ps_div", bufs=1, space="PSUM") as ps_div, \
             tc.tile_pool(name="ps_strm", bufs=2, space="PSUM") as ps_strm:

            maskt = {}

            # per (s, head-pair) division inputs live one hp iteration
            div_in = {}

            def attn_hp(s, hp, jit_hook=None):
                avp = [ps_av.tile([65, 512], F32, tag="av", name=f"av_{s}_{hp}_{hh}")
                       for hh in range(2)]
                for j in range(NKV[s]):
                    if jit_hook is not None:
                        jit_hook(j)
                    for hh in range(2):
                        lo = hh * 64
                        sp = ps_sc.tile([P, 512], F32, tag="sc", name=f"sc_{s}_{hp}_{j}_{hh}")
                        nc.tensor.matmul(
                            sp[:], kt_t[hp][j // 4][lo:lo + 64, (j % 4) * P:(j % 4) * P + P],
                            qt_t[hp][s][lo:lo + 64, :], start=True, stop=True)
                        es_ = p_es.tile([P, 512], BF16, tag="es", name=f"es_{s}_{hp}_{j}_{hh}")
                        nc.scalar.activation(es_[:], sp[:], AF.Exp, scale=HD ** -0.5)
                        if _masked(s, j):
                            nc.vector.tensor_mul(es_[:], es_[:], maskt[j][:])
                        if DBG and s == 0 and hp == 0 and j == 0 and hh == 0:
                            nc.sync.dma_start(d["dbg"][10], es_[:])
                        nc.tensor.matmul(
                            avp[hh][:],
                            v_t[j].rearrange("p (h c) -> p h c", c=65)[:, 2 * hp + hh],
                            es_[:], start=(j == 0), stop=(j == NKV[s] - 1))
                # unload raw + reciprocal of denominator (DVE, off critical path)
                raws, rhbs = [], []
                for hh in range(2):
                    raw = p_raw.tile([64, 512], BF16, tag="raw", name=f"rw_{s}_{hp}_{hh}")
                    nc.vector.tensor_copy(raw[:], avp[hh][0:64, :])
                    den = p_rec.tile([65, 512], F32, tag="den", name=f"dn_{s}_{hp}_{hh}")
                    nc.vector.tensor_copy(den[64:65, :], avp[hh][64:65, :])
                    rec = p_rec.tile([65, 512], F32, tag="rec", name=f"rc_{s}_{hp}_{hh}")
                    nc.vector.reciprocal(rec[64:65, :], den[64:65, :])
                    rhb = p_rhb.tile([65, 512], BF16, tag="rhb", name=f"rh_{s}_{hp}_{hh}")
                    nc.vector.tensor_copy(rhb[64:65, :], rec[64:65, :])
                    if DBG and s == 0 and hp == 0:
                        nc.sync.dma_start(d["dbg"][11, hh:hh + 1, :], rhb[64:65, :])
                        nc.sync.dma_start(d["dbg"][11, 2 + hh:3 + hh, :], raw[0:1, :])
                    raws.append(raw)
                    rhbs.append(rhb)
                div_in[(s, hp)] = (raws, rhbs)

            def attn_div(s, hp):
                raws, rhbs = div_in.pop((s, hp))
                rbe = ps_div.tile([P, 512], F32, tag="rb", name=f"rbe_{s}_{hp}")
                nc.tensor.matmul(rbe[0:64, :], ones[64:65, 0:64], rhbs[0][64:65, :],
                                 start=True, stop=True)
                rbo = ps_div.tile([P, 512], F32, tag="ap", name=f"rbo_{s}_{hp}")
                nc.tensor.matmul(rbo[0:64, :], ones[64:65, 0:64], rhbs[1][64:65, :],
                                 start=True, stop=True)
                sc1 = p_sc1.tile([64, 512], BF16, tag="sc1", name=f"sm_{s}_{hp}")
                nc.vector.tensor_mul(sc1[:], raws[1][:], rbo[0:64, :])
                nc.tensor.matmul(rbo[64:128, :], ident[:], sc1[:],
                                 start=True, stop=True, skip_group_check=True)
                nc.vector.tensor_mul(att_t[hp][s][0:64, :], raws[0][:], rbe[0:64, :])
                nc.vector.tensor_copy(att_t[hp][s][64:128, :], rbo[64:128, :])

            # ---------------- P2: attention s0 + fillers ----------------
            with tc.tile_pool(name="p_wvs", bufs=1) as p_wvs, \
                 tc.tile_pool(name="p_wsl2", bufs=2) as p_wsl2, \
                 tc.tile_pool(name="p_msk0", bufs=1) as p_msk0:

                for mi in range(8):
                    mt = p_msk0.tile([P, 512], BF16, tag=f"m{mi}", name=f"mask{mi}")
                    nc.sync.dma_start(mt[:], d["masks"][mi])
                    maskt[mi] = mt
                wvs = [[p_wvs.tile([P, 512], BF16, tag=f"wv{di}_{doc}", name=f"wvs_{di}_{doc}")
                        for doc in range(2)] for di in range(DC)]
                for di in range(DC):
                    for doc in range(2):
                        nc.sync.dma_start(wvs[di][doc][:],
                                          d["wv"][di, :, doc * 512:(doc + 1) * 512])

                def v_proj(j):
                    t5, jo = j // 4, (j % 4) * P
                    for doc in range(2):
                        pt = ps_strm.tile([P, 512], F32, tag="strm", name=f"pv_{j}_{doc}")
                        for di in range(DC):
                            nc.tensor.matmul(pt[:], h_t[di][t5][:, jo:jo + P],
                                             wvs[di][doc][:],
                                             start=(di == 0), stop=(di == DC - 1))
                        dst = v_t[j].rearrange("p (h c) -> p h c", c=65)[:, doc * 8:(doc + 1) * 8, 0:64]
                        nc.scalar.copy(dst, pt.rearrange("p (h c) -> p h c", c=64))
                    nc.scalar.activation(
                        v_t[j].rearrange("p (h c) -> p h c", c=65)[:, :, 64:65],
                        ones[:, 0:H].unsqueeze(2), AF.Copy)

                def kq_unit(dc):
                    wsl = p_wsl2.tile([P, DC, P], BF16, tag="wsl2", name=f"wks2_{dc}")
                    nc.sync.dma_start(wsl[:], d["wk"][dc])
                    for t in (2, 3):
                        pt = ps_strm.tile([P, 512], F32, tag="strm", name=f"p2k_{dc}_{t}")
                        for di in range(DC):
                            nc.tensor.matmul(pt[:], wsl[:, di], h_t[di][t][:],
                                             start=(di == 0), stop=(di == DC - 1))
                        nc.scalar.copy(kt_t[dc][t][:], pt[:])

                def q_unit(dc):
                    wsl = p_wsl2.tile([P, DC, P], BF16, tag="wsl2", name=f"wqs2_{dc}")
                    nc.sync.dma_start(wsl[:], d["wq"][dc])
                    pt = ps_strm.tile([P, 512], F32, tag="strm", name=f"p2q_{dc}")
                    for di in range(DC):
                        nc.tensor.matmul(pt[:], wsl[:, di], hq_t[di][1][:],
                                         start=(di == 0), stop=(di == DC - 1))
                    nc.scalar.copy(qt_t[dc][1][:], pt[:])

                v_proj(0)
                v_proj(1)

                def jit_v(j):
                    if j + 2 < 8:
                        v_proj(j + 2)

                fillers2 = [lambda j=j: v_proj(j) for j in range(8, 16)]
                fi2 = 0
                for hp in range(DC):
                    attn_hp(0, hp, jit_hook=(jit_v if hp == 0 else None))
                    if hp > 0:
                        attn_div(0, hp - 1)
                    if hp > 0:
                        for _ in range(2):
                            if fi2 < len(fillers2):
                                fillers2[fi2]()
                                fi2 += 1
                while fi2 < len(fillers2):
                    fillers2[fi2]()
                    fi2 += 1
                attn_div(0, DC - 1)
                for dc in range(DC):
                    kq_unit(dc)
                for dc in range(DC):
                    q_unit(dc)

            if DBG:
                nc.sync.dma_start(d["dbg"][1], h_t[7][3][:])
                nc.sync.dma_start(d["dbg"][6], kt_t[0][3][:])
                nc.sync.dma_start(d["dbg"][7], qt_t[0][1][:])
                nc.sync.dma_start(d["dbg"][8], v_t[0][:, 0:512])
                nc.sync.dma_start(d["dbg"][9], v_t[15][:, 0:512])
                nc.sync.dma_start(d["dbg"][12], att_t[0][0][:])
                nc.sync.dma_start(d["dbg"][16], kt_t[0][2][:])
                nc.sync.dma_start(d["dbg"][17], kt_t[3][3][:])
                nc.sync.dma_start(d["dbg"][18], v_t[8][:, 0:512])
                nc.sync.dma_start(d["dbg"][19], v_t[2][:, 0:512])

            # slot-1 masks reuse the dead qt(slot0) tile slots in p_main
            for i in range(8):
                mt = p_main.tile([P, 512], BF16, tag=f"pc{2 * i}", name=f"mask{8 + i}")
                nc.sync.dma_start(mt[:], d["masks"][8 + i])
                maskt[8 + i] = mt

            # ---------------- P3 + P4 scope ----------------
            with tc.tile_pool(name="p_wo", bufs=1) as p_wo, \
                 tc.tile_pool(name="p_w1s", bufs=2) as p_w1s, \
                 tc.tile_pool(name="p_w2s", bufs=2) as p_w2s, \
                 tc.tile_pool(name="p_xqf", bufs=2) as p_xqf, \
                 tc.tile_pool(name="p_f2", bufs=1) as p_f2, \
                 tc.tile_pool(name="p_px", bufs=1) as p_px, \
                 tc.tile_pool(name="p_rows2", bufs=3) as p_rows2, \
                 tc.tile_pool(name="p_tmp2", bufs=2) as p_tmp2, \
                 tc.tile_pool(name="p_sq2", bufs=1) as p_sq2:

                x2_t = {}
                for s in range(S):
                    for dc in range(DC):
                        x2_t[(dc, s)] = p_px.tile([P, 512], F32, tag=f"px{dc}",
                                                  name=f"x2_{dc}_{s}")
                xb2_t, h2_t = {}, {}
                for dc in range(DC):
                    xb2_t[(dc, 0)] = p_main.tile([P, 512], BF16, tag=f"pa{32 + dc * S}",
                                                 name=f"xb2_{dc}_0")
                    h2_t[(dc, 0)] = p_main.tile([P, 512], BF16, tag=f"pa{32 + dc * S + 1}",
                                                name=f"h2_{dc}_0")
                    xb2_t[(dc, 1)] = p_main.tile([P, 512], BF16, tag=f"pb{dc * TCKV}",
                                                 name=f"xb2_{dc}_1")
                    h2_t[(dc, 1)] = p_main.tile([P, 512], BF16, tag=f"pb{dc * TCKV + 1}",
                                                name=f"h2_{dc}_1")
                ff1_t = [p_main.tile([P, 512], BF16, tag=f"pa{fc}", name=f"ff1_{fc}")
                         for fc in range(FC)]

                def o_proj(s, dc):
                    wosl = p_wo.tile([P, DC, P], BF16, tag="wo", name=f"wos_{dc}_{s}")
                    nc.sync.dma_start(wosl[:], d["wo"][dc])
                    pt = ps_strm.tile([P, 512], F32, tag="strm", name=f"po_{dc}_{s}")
                    for di in range(DC):
                        nc.tensor.matmul(pt[:], wosl[:, di], att_t[di][s][:],
                                         start=(di == 0), stop=(di == DC - 1))
                    x2t = x2_t[(dc, s)]
                    nc.scalar.activation(x2t[:], pt[:], AF.Identity, bias=bo_col(dc))
                    xqf = p_xqf.tile([P, 512], F32, tag="xqf", name=f"xqf_{dc}_{s}")
                    nc.sync.dma_start(xqf[:], d["xqf"][dc, s])
                    nc.vector.tensor_add(x2t[:], x2t[:], xqf[:])
                    nc.vector.tensor_copy(xb2_t[(dc, s)][:], x2t[:])
                    if DBG and s == 0 and dc == 0:
                        nc.sync.dma_start(d["dbg"][13], xb2_t[(dc, s)][:])

                def ln2_stats(s):
                    xbt = [xb2_t[(dc, s)] for dc in range(DC)]
                    st1 = ps_strm.tile([P, 512], F32, tag="strm", name=f"c2s1_{s}")
                    sqs = []
                    for dc in range(DC):
                        sq = p_sq2.tile([P, 512], BF16, tag="sq2", name=f"c2sq_{dc}_{s}")
                        nc.vector.tensor_mul(sq[:], xbt[dc][:], xbt[dc][:])
                        sqs.append(sq)
                    for dc in range(DC):
                        nc.tensor.matmul(st1[0:1, :], ones[:, 0:1], xbt[dc][:],
                                         start=(dc == 0), stop=(dc == DC - 1))
                    st2 = ps_strm.tile([P, 512], F32, tag="strm", name=f"c2s2_{s}")
                    for dc in range(DC):
                        nc.tensor.matmul(st2[0:1, :], ones[:, 0:1], sqs[dc][:],
                                         start=(dc == 0), stop=(dc == DC - 1))
                    mu = p_rows2.tile([1, 512], F32, tag="rf2", name=f"c2mu_{s}")
                    nc.vector.tensor_scalar_mul(mu[:], st1[0:1, :], 1.0 / D)
                    msq = p_rows2.tile([1, 512], F32, tag="rf2", name=f"c2ms_{s}")
                    nc.vector.tensor_scalar_mul(msq[:], st2[0:1, :], 1.0 / D)
                    var = p_rows2.tile([1, 512], F32, tag="rf2", name=f"c2va_{s}")
                    nc.vector.tensor_mul(var[:], mu[:], mu[:])
                    nc.vector.tensor_sub(var[:], msq[:], var[:])
                    sd = p_rows2.tile([1, 512], F32, tag="rf2", name=f"c2sd_{s}")
                    nc.scalar.activation(sd[:], var[:], AF.Sqrt, bias=epst[:])
                    rsig = p_rows2.tile([1, 512], F32, tag="rf2", name=f"c2rs_{s}")
                    nc.vector.reciprocal_approx_fast(out=rsig[:], in_=sd[:])
                    cmu = p_rows2.tile([1, 512], F32, tag="rf2", name=f"c2cm_{s}")
                    nc.vector.tensor_mul(cmu[:], mu[:], rsig[:])
                    ah = p_rows2.tile([1, 512], BF16, tag="rb2", name=f"c2ah_{s}")
                    nc.vector.tensor_copy(ah[:], rsig[:])
                    ch = p_rows2.tile([1, 512], BF16, tag="rb2", name=f"c2ch_{s}")
                    nc.vector.tensor_copy(ch[:], cmu[:])
                    return (ah, ch)

                def ln2_apply(s, rows):
                    ah, ch = rows
                    bcA = ps_strm.tile([P, 512], F32, tag="strm", name=f"c2bA_{s}")
                    nc.tensor.matmul(bcA[:], ones[0:1, :], ah[:], start=True, stop=True)
                    bcC = ps_strm.tile([P, 512], F32, tag="strm", name=f"c2bC_{s}")
                    nc.tensor.matmul(bcC[:], ones[0:1, :], ch[:], start=True, stop=True)
                    for dc in range(DC):
                        tmp = p_tmp2.tile([P, 512], F32, tag="lnt2", name=f"c2lt_{dc}_{s}")
                        nc.vector.tensor_mul(tmp[:], xb2_t[(dc, s)][:], bcA[:])
                        nc.vector.tensor_sub(tmp[:], tmp[:], bcC[:])
                        nc.scalar.activation(h2_t[(dc, s)][:], tmp[:], AF.Identity,
                                             bias=bl2_col(dc), scale=g2_col(dc))
                    if DBG and s == 0:
                        nc.sync.dma_start(d["dbg"][14], h2_t[(0, s)][:])

                def ffn1(s, fc):
                    w1s = p_w1s.tile([P, DC, P], BF16, tag="w1s", name=f"w1s_{s}_{fc}")
                    nc.sync.dma_start(w1s[:], d["w1"][fc])
                    pt = ps_strm.tile([P, 512], F32, tag="strm", name=f"pf_{fc}_{s}")
                    for di in range(DC):
                        nc.tensor.matmul(pt[:], w1s[:, di], h2_t[(di, s)][:],
                                         start=(di == 0), stop=(di == DC - 1))
                    nc.scalar.activation(ff1_t[fc][:], pt[:], AF.Relu,
                                         bias=b1t[:, fc:fc + 1])
                    if DBG and s == 0 and fc == 0:
                        nc.sync.dma_start(d["dbg"][15], ff1_t[fc][:])

                def ffn2(s, dc):
                    QW = FC // 4
                    pt = ps_strm.tile([P, 512], F32, tag="strm", name=f"pg_{dc}_{s}")
                    for qi in range(4):
                        w2q = p_w2s.tile([P, QW, P], BF16, tag="w2s", name=f"w2q_{s}_{dc}_{qi}")
                        nc.sync.dma_start(w2q[:], d["w2"][dc, :, qi * QW:(qi + 1) * QW])
                        for fi in range(QW):
                            fc = qi * QW + fi
                            nc.tensor.matmul(pt[:], w2q[:, fi], ff1_t[fc][:],
                                             start=(fc == 0), stop=(fc == FC - 1))
                    f2 = p_f2.tile([P, 512], F32, tag="f2", name=f"f2_{dc}_{s}")
                    nc.scalar.activation(f2[:], pt[:], AF.Relu, bias=b2_col(dc))
                    nc.vector.tensor_add(f2[:], f2[:], x2_t[(dc, s)][:])
                    nc.sync.dma_start(d["outT"][dc, s], f2[:])

                # ---------------- P3: attention s1 + s0 tail ----------------
                ln2_rows = {}
                fillers3 = ([lambda dc=dc: o_proj(0, dc) for dc in range(DC)]
                            + [lambda: ln2_rows.__setitem__(0, ln2_stats(0)),
                               lambda: ln2_apply(0, ln2_rows.pop(0))]
                            + [lambda fc=fc: ffn1(0, fc) for fc in range(FC)]
                            + [lambda dc=dc: ffn2(0, dc) for dc in range(DC)])
                fi3 = 0
                for hp in range(DC):
                    attn_hp(1, hp)
                    if hp > 0:
                        attn_div(1, hp - 1)
                    for _ in range(7):
                        if fi3 < len(fillers3):
                            fillers3[fi3]()
                            fi3 += 1
                attn_div(1, DC - 1)
                while fi3 < len(fillers3):
                    fillers3[fi3]()
                    fi3 += 1

                # ---------------- P4: s1 tail ----------------
                for dc in range(DC):
                    o_proj(1, dc)
                rows1 = ln2_stats(1)
                ln2_apply(1, rows1)
                for fc in range(FC):
                    ffn1(1, fc)
                for dc in range(DC):
                    ffn2(1, dc)


# ============================ host side ============================

def _slab(w, rows_chunks, cols_chunks):
    r, c = w.shape
    return np.ascontiguousarray(
        w.reshape(rows_chunks, r // rows_chunks, cols_chunks, c // cols_chunks)
        .transpose(2, 1, 0, 3)).astype(ml_dtypes.bfloat16)


def _prep_core(inputs, core):
    b, p = core // 2, core % 2
    bf16 = ml_dtypes.bfloat16
    x = np.asarray(inputs["x"], np.float32)[b]
    xT = np.ascontiguousarray(x.T)
    qb = QBLOCKS[p]
    qidx = np.concatenate([np.arange(q_ * 512, q_ * 512 + 512) for q_ in qb])
    xqT = np.ascontiguousarray(xT[:, qidx])

    m = {}
    m["xkv"] = np.ascontiguousarray(
        xT.reshape(DC, P, TCKV, 512).transpose(0, 2, 1, 3)).astype(bf16)
    xq4 = np.ascontiguousarray(xqT.reshape(DC, P, S, 512).transpose(0, 2, 1, 3))
    m["xqb"] = xq4.astype(bf16)
    m["xqf"] = xq4.astype(np.float32)
    m["wq"] = _slab(np.asarray(inputs["Wq"], np.float32), DC, DC)
    m["wk"] = _slab(np.asarray(inputs["Wk"], np.float32), DC, DC)
    m["wo"] = _slab(np.asarray(inputs["Wo"], np.float32), DC, DC)
    m["wv"] = np.ascontiguousarray(
        np.asarray(inputs["Wv"], np.float32).reshape(DC, P, D)).astype(bf16)
    m["w1"] = _slab(np.asarray(inputs["W1"], np.float32), DC, FC)
    m["w2"] = _slab(np.asarray(inputs["W2"], np.float32), FC, DC)

    masks = np.zeros((NMASK, P, 512), np.float32)
    for s in range(S):
        qstart = qb[s] * 512
        for j in (range(8) if s == 0 else range(8, 16)):
            kv = j * P + np.arange(P)[:, None]
            qg = qstart + np.arange(512)[None, :]
            masks[j] = (kv <= qg).astype(np.float32)
    m["masks"] = masks.astype(bf16)

    biasg = np.zeros((P, 6 * DC), np.float32)
    for i, key in enumerate(["bo", "ln1_g", "ln1_b", "ln2_g", "ln2_b", "b2"]):
        biasg[:, i * DC:(i + 1) * DC] = np.asarray(inputs[key], np.float32).reshape(DC, P).T
    m["biasg"] = np.ascontiguousarray(biasg)
    m["b1c"] = np.ascontiguousarray(
        np.asarray(inputs["b1"], np.float32).reshape(FC, P).T)
    m["onesc"] = np.ones((P, P), bf16)
    m["ident"] = np.eye(64, dtype=np.float32).astype(bf16)
    m["epsv"] = np.full((1, 1), EPS, np.float32)
    return m


def kernel(**inputs):
    if "nc" not in _built:
        _built["nc"] = build_nc()
    nc = _built["nc"]
    in_maps = [_prep_core(inputs, c) for c in range(8)]
    res = run_bass_kernel_spmd(nc, in_maps, core_ids=list(range(8)))
    out = np.zeros((B, T, D), np.float32)
    for c in range(8):
        b, p = c // 2, c % 2
        o = np.asarray(res.results[c]["outT"])
        for s in range(S):
            qb = QBLOCKS[p][s]
            blk = o[:, s].reshape(D, 512)
            out[b, qb * 512:(qb + 1) * 512, :] = blk.T
    return out.astype(np.float32)


# revision 48
# speedup vs baseline: 1.1911x; 1.0827x over previous
"""Trainium2 Bass kernel for a pre-LN transformer block (B=4, T=2048, D=1024, H=16).

Sharding: 8 cores = (batch b = core//2) x (half p = core%2). Each core handles
1024 query tokens of its batch: p=0 -> 512-token blocks {0,3}, p=1 -> {1,2}
(balanced causal work). K/V are recomputed per core from the full batch
sequence (no collectives). Per-core variation (token selection, causal masks)
is carried entirely in input data so one uniform SPMD program serves all cores.

Layout: feature-major ("transposed") activations [D, tokens] so every matmul
uses weights as stored (lhsT = W chunk), attention scores/AV need no on-chip
transposes, and softmax denominators come from a ones-column appended to V.

v2 scheduling: software-pipelined so the PE never idles long enough for the
HAM clock gate to re-throttle it:
  P1: LN1 + Q(slot0) + K(t0,t1)
  P2: attention slot0 (8 kv blocks) + fillers: V(all 16), K(t2,t3), Q(slot1)
  P3: attention slot1 (16 kv blocks) + fillers: O/LN2/FFN of slot0
  P4: O/LN2/FFN of slot1
Divisions run one head-pair behind their attention loop. Softmax reciprocals
use reciprocal_approx_fast in-place in PSUM; LN uses a single ACT Rsqrt; x2
residuals round-trip through DRAM scratch to stay under the SBUF budget.
"""

import sys

sys.path.insert(0, "/opt/trn_rl_repo")

import numpy as np
import ml_dtypes

import concourse.bass as bass
import concourse.mybir as mybir
import concourse.tile as tile
from concourse import bacc
from concourse.bass_utils import run_bass_kernel_spmd

BF16 = mybir.dt.bfloat16
F32 = mybir.dt.float32
AF = mybir.ActivationFunctionType

B, T, D, H, HD = 4, 2048, 1024, 16, 64
EPS = 1e-5
P = 128
DC = D // P            # 8 feature chunks
S = 2                  # q slots per core (512 tokens each)
TCKV = T // 512        # 4 kv token 512-chunks
NKV = [8, 16]          # kv 128-blocks per slot (uniform across cores)
FC = 4 * D // P        # 32 ffn hidden chunks
NMASK = 16
QBLOCKS = [[0, 3], [1, 2]]

_built = {}
DBG = False  # when True, adds a "dbg" output tensor with intermediate dumps


def _masked(s, j):
    return (s == 0) or (j >= 8)


def build_nc():
    nc = bacc.Bacc("TRN2", target_bir_lowering=False, debug=False, num_devices=8)

    d = {}
    d["xkv"] = nc.dram_tensor("xkv", [DC, TCKV, P, 512], BF16, kind="ExternalInput").ap()
    d["xqb"] = nc.dram_tensor("xqb", [DC, S, P, 512], BF16, kind="ExternalInput").ap()
    d["xqf"] = nc.dram_tensor("xqf", [DC, S, P, 512], F32, kind="ExternalInput").ap()
    d["wq"] = nc.dram_tensor("wq", [DC, P, DC, P], BF16, kind="ExternalInput").ap()
    d["wk"] = nc.dram_tensor("wk", [DC, P, DC, P], BF16, kind="ExternalInput").ap()
    d["wo"] = nc.dram_tensor("wo", [DC, P, DC, P], BF16, kind="ExternalInput").ap()
    d["wv"] = nc.dram_tensor("wv", [DC, P, D], BF16, kind="ExternalInput").ap()
    d["w1"] = nc.dram_tensor("w1", [FC, P, DC, P], BF16, kind="ExternalInput").ap()
    d["w2"] = nc.dram_tensor("w2", [DC, P, FC, P], BF16, kind="ExternalInput").ap()
    d["masks"] = nc.dram_tensor("masks", [NMASK, P, 512], BF16, kind="ExternalInput").ap()
    d["biasg"] = nc.dram_tensor("biasg", [P, 6 * DC], F32, kind="ExternalInput").ap()
    d["b1c"] = nc.dram_tensor("b1c", [P, FC], F32, kind="ExternalInput").ap()
    d["onesc"] = nc.dram_tensor("onesc", [P, P], BF16, kind="ExternalInput").ap()
    d["ident"] = nc.dram_tensor("ident", [64, 64], BF16, kind="ExternalInput").ap()
    d["epsv"] = nc.dram_tensor("epsv", [1, 1], F32, kind="ExternalInput").ap()
    if DBG:
        d["dbg"] = nc.dram_tensor("dbg", [20, P, 512], BF16, kind="ExternalOutput").ap()
    d["outT"] = nc.dram_tensor("outT", [DC, S, P, 512], F32, kind="ExternalOutput").ap()

    with tile.TileContext(nc) as tc:
        _emit(nc, tc, d)
    nc.compile()
    return nc


def _emit(nc, tc, d):
    from contextlib import ExitStack

    with ExitStack() as es:
        consts = es.enter_context(tc.tile_pool(name="consts", bufs=1))

        ones = consts.tile([P, P], BF16, tag="ones", name="ones")
        nc.sync.dma_start(ones[:], d["onesc"][:])
        ident = consts.tile([64, 64], BF16, tag="ident", name="ident")
        nc.sync.dma_start(ident[:], d["ident"][:])
        biasg = consts.tile([P, 6 * DC], F32, tag="biasg", name="biasg")
        nc.sync.dma_start(biasg[:], d["biasg"][:])
        b1t = consts.tile([P, FC], F32, tag="b1t", name="b1t")
        nc.sync.dma_start(b1t[:], d["b1c"][:])
        epst = consts.tile([1, 1], F32, tag="epst", name="epst")
        nc.sync.dma_start(epst[:], d["epsv"][:])

        bo_col = lambda dc: biasg[:, dc:dc + 1]
        g1_col = lambda dc: biasg[:, DC + dc:DC + dc + 1]
        bl1_col = lambda dc: biasg[:, 2 * DC + dc:2 * DC + dc + 1]
        g2_col = lambda dc: biasg[:, 3 * DC + dc:3 * DC + dc + 1]
        bl2_col = lambda dc: biasg[:, 4 * DC + dc:4 * DC + dc + 1]
        b2_col = lambda dc: biasg[:, 5 * DC + dc:5 * DC + dc + 1]

        # ---------- persistent tiles (tags reused across disjoint lifetimes) --
        # pa0..31: h (P1-P2) then ff1 (P3: s0, P4: s1)
        # pa32..47: hq (P1-P2) then xb2/h2 of s0 (P3+)
        # pb0..31: kt (P1-P3); pb{4dc},pb{4dc+1} then xb2/h2 of s1 (P4)
        # pv0..15: v;  pc0..15: qt;  pt0..15: att
        p_main = es.enter_context(tc.tile_pool(name="p_main", bufs=1))

        h_t = [[p_main.tile([P, 512], BF16, tag=f"pa{dc * TCKV + t}", name=f"h_{dc}_{t}")
                for t in range(TCKV)] for dc in range(DC)]
        hq_t = [[p_main.tile([P, 512], BF16, tag=f"pa{32 + dc * S + s}", name=f"hq_{dc}_{s}")
                 for s in range(S)] for dc in range(DC)]
        kt_t = [[p_main.tile([P, 512], BF16, tag=f"pb{dc * TCKV + t}", name=f"kt_{dc}_{t}")
                 for t in range(TCKV)] for dc in range(DC)]
        v_t = [p_main.tile([P, H * 65], BF16, tag=f"pv{j}", name=f"v_{j}")
               for j in range(T // P)]
        qt_t = [[p_main.tile([P, 512], BF16, tag=f"pc{dc * S + s}", name=f"qt_{dc}_{s}")
                 for s in range(S)] for dc in range(DC)]
        att_t = [[p_main.tile([P, 512], BF16, tag=f"pt{dc * S + s}", name=f"at_{dc}_{s}")
                  for s in range(S)] for dc in range(DC)]

        # =================== P1: LN1 + Q(s0) + K(t0,t1) =====================
        with tc.tile_pool(name="p_xsrc", bufs=16) as p_xsrc, \
             tc.tile_pool(name="p_sq", bufs=8) as p_sq, \
             tc.tile_pool(name="p_rows", bufs=4) as p_rows, \
             tc.tile_pool(name="p_tmp", bufs=4) as p_tmp, \
             tc.tile_pool(name="p_wsl", bufs=2) as p_wsl, \
             tc.tile_pool(name="ps_st", bufs=2, space="PSUM") as ps_st, \
             tc.tile_pool(name="ps_bc", bufs=2, space="PSUM") as ps_bc, \
             tc.tile_pool(name="ps_mm", bufs=4, space="PSUM") as ps_mm:

            def ln_stats(ci, src_fn, pfx):
                xbt = src_fn()
                st1 = ps_st.tile([P, 512], F32, tag="st", name=f"{pfx}s1_{ci}")
                st2 = ps_st.tile([P, 512], F32, tag="st", name=f"{pfx}s2_{ci}")
                sqs = []
                for dc in range(DC):
                    sq = p_sq.tile([P, 512], BF16, tag="sq", name=f"{pfx}sq_{dc}_{ci}")
                    nc.vector.tensor_mul(sq[:], xbt[dc][:], xbt[dc][:])
                    sqs.append(sq)
                for dc in range(DC):
                    nc.tensor.matmul(st1[0:1, :], ones[:, 0:1], xbt[dc][:],
                                     start=(dc == 0), stop=(dc == DC - 1))
                for dc in range(DC):
                    nc.tensor.matmul(st2[0:1, :], ones[:, 0:1], sqs[dc][:],
                                     start=(dc == 0), stop=(dc == DC - 1))
                mu = p_rows.tile([1, 512], F32, tag="rf", name=f"{pfx}mu_{ci}")
                nc.vector.tensor_scalar_mul(mu[:], st1[0:1, :], 1.0 / D)
                msq = p_rows.tile([1, 512], F32, tag="rf", name=f"{pfx}ms_{ci}")
                nc.vector.tensor_scalar_mul(msq[:], st2[0:1, :], 1.0 / D)
                var = p_rows.tile([1, 512], F32, tag="rf", name=f"{pfx}va_{ci}")
                nc.vector.tensor_mul(var[:], mu[:], mu[:])
                nc.vector.tensor_sub(var[:], msq[:], var[:])
                sd = p_rows.tile([1, 512], F32, tag="rf", name=f"{pfx}sd_{ci}")
                nc.scalar.activation(sd[:], var[:], AF.Sqrt, bias=epst[:])
                rsig = p_rows.tile([1, 512], F32, tag="rf", name=f"{pfx}rs_{ci}")
                nc.vector.reciprocal_approx_fast(out=rsig[:], in_=sd[:])
                cmu = p_rows.tile([1, 512], F32, tag="rf", name=f"{pfx}cm_{ci}")
                nc.vector.tensor_mul(cmu[:], mu[:], rsig[:])
                ah = p_rows.tile([1, 512], BF16, tag="rb", name=f"{pfx}ah_{ci}")
                nc.vector.tensor_copy(ah[:], rsig[:])
                ch = p_rows.tile([1, 512], BF16, tag="rb", name=f"{pfx}ch_{ci}")
                nc.vector.tensor_copy(ch[:], cmu[:])
                return xbt, (ah, ch)

            def ln_apply(ci, xbt, rows, h_tiles, pfx):
                ah, ch = rows
                bcA = ps_bc.tile([P, 512], F32, tag="bc", name=f"{pfx}bA_{ci}")
                nc.tensor.matmul(bcA[:], ones[0:1, :], ah[:], start=True, stop=True)
                bcC = ps_bc.tile([P, 512], F32, tag="bc", name=f"{pfx}bC_{ci}")
                nc.tensor.matmul(bcC[:], ones[0:1, :], ch[:], start=True, stop=True)
                for dc in range(DC):
                    tmp = p_tmp.tile([P, 512], F32, tag="lntmp", name=f"{pfx}lt_{dc}_{ci}")
                    nc.vector.tensor_mul(tmp[:], xbt[dc][:], bcA[:])
                    nc.vector.tensor_sub(tmp[:], tmp[:], bcC[:])
                    nc.scalar.activation(h_tiles[dc][:], tmp[:], AF.Identity,
                                         bias=bl1_col(dc), scale=g1_col(dc))

            def mk_src_kv(tcx):
                def f():
                    out = []
                    for dc in range(DC):
                        xt = p_xsrc.tile([P, 512], BF16, tag="xsrc", name=f"xkv_{dc}_{tcx}")
                        nc.sync.dma_start(xt[:], d["xkv"][dc, tcx])
                        out.append(xt)
                    return out
                return f

            def mk_src_q(s):
                def f():
                    out = []
                    for dc in range(DC):
                        xt = p_xsrc.tile([P, 512], BF16, tag="xsrc", name=f"xqb_{dc}_{s}")
                        nc.sync.dma_start(xt[:], d["xqb"][dc, s])
                        out.append(xt)
                    return out
                return f

            def q_proj1():
                for dc in range(DC):
                    wsl = p_wsl.tile([P, DC, P], BF16, tag="wsl", name=f"wqs_{dc}")
                    nc.sync.dma_start(wsl[:], d["wq"][dc])
                    pt = ps_mm.tile([P, 512], F32, tag="mm", name=f"pq_{dc}_0")
                    for di in range(DC):
                        nc.tensor.matmul(pt[:], wsl[:, di], hq_t[di][0][:],
                                         start=(di == 0), stop=(di == DC - 1))
                    nc.vector.tensor_copy(qt_t[dc][0][:], pt[:])

            def k_proj1():
                for dc in range(DC):
                    wsl = p_wsl.tile([P, DC, P], BF16, tag="wsl", name=f"wks_{dc}")
                    nc.sync.dma_start(wsl[:], d["wk"][dc])
                    for t in range(2):
                        pt = ps_mm.tile([P, 512], F32, tag="mm", name=f"pk_{dc}_{t}")
                        for di in range(DC):
                            nc.tensor.matmul(pt[:], wsl[:, di], h_t[di][t][:],
                                             start=(di == 0), stop=(di == DC - 1))
                        nc.vector.tensor_copy(kt_t[dc][t][:], pt[:])

            # chunk order: qS, t0, t1, qB, t2, t3; apply lags one chunk behind
            chunks = [(mk_src_q(0), [hq_t[dc][0] for dc in range(DC)], "aq0"),
                      (mk_src_kv(0), [h_t[dc][0] for dc in range(DC)], "at0"),
                      (mk_src_kv(1), [h_t[dc][1] for dc in range(DC)], "at1"),
                      (mk_src_q(1), [hq_t[dc][1] for dc in range(DC)], "aq1"),
                      (mk_src_kv(2), [h_t[dc][2] for dc in range(DC)], "at2"),
                      (mk_src_kv(3), [h_t[dc][3] for dc in range(DC)], "at3")]
            pend = None
            post = {1: q_proj1, 3: k_proj1}
            for ci, (src_fn, h_tiles, pfx) in enumerate(chunks):
                xbt, rows = ln_stats(ci, src_fn, pfx)
                if pend is not None:
                    ln_apply(pend[0], pend[1], pend[2], pend[3], pend[4])
                pend = (ci, xbt, rows, h_tiles, pfx)
                if ci in post:
                    post[ci]()
            ln_apply(pend[0], pend[1], pend[2], pend[3], pend[4])
            if DBG:
                nc.sync.dma_start(d["dbg"][0], h_t[0][0][:])
                nc.sync.dma_start(d["dbg"][2], hq_t[0][0][:])
                nc.sync.dma_start(d["dbg"][3], hq_t[0][1][:])
                nc.sync.dma_start(d["dbg"][4], qt_t[0][0][:])
                nc.sync.dma_start(d["dbg"][5], kt_t[0][0][:])

        # =================== attention-wide scope (P2..P4) ==================
        with tc.tile_pool(name="p_es", bufs=3) as p_es, \
             tc.tile_pool(name="p_raw", bufs=4) as p_raw, \
             tc.tile_pool(name="p_rhb", bufs=4) as p_rhb, \
             tc.tile_pool(name="p_rec", bufs=1) as p_rec, \
             tc.tile_pool(name="p_sc1", bufs=1) as p_sc1, \
             tc.tile_pool(name="ps_sc", bufs=2, space="PSUM") as ps_sc, \
             tc.tile_pool(name="ps_av", bufs=2, space="PSUM") as ps_av, \
             tc.tile_pool(name="ps_div", bufs=1, space="PSUM") as ps_div, \
             tc.tile_pool(name="ps_strm", bufs=2, space="PSUM") as ps_strm:

            maskt = {}

            # per (s, head-pair) division inputs live one hp iteration
            div_in = {}

            def attn_hp(s, hp, jit_hook=None):
                avp = [ps_av.tile([65, 512], F32, tag="av", name=f"av_{s}_{hp}_{hh}")
                       for hh in range(2)]
                for j in range(NKV[s]):
                    if jit_hook is not None:
                        jit_hook(j)
                    for hh in range(2):
                        lo = hh * 64
                        sp = ps_sc.tile([P, 512], F32, tag="sc", name=f"sc_{s}_{hp}_{j}_{hh}")
                        nc.tensor.matmul(
                            sp[:], kt_t[hp][j // 4][lo:lo + 64, (j % 4) * P:(j % 4) * P + P],
                            qt_t[hp][s][lo:lo + 64, :], start=True, stop=True)
                        es_ = p_es.tile([P, 512], BF16, tag="es", name=f"es_{s}_{hp}_{j}_{hh}")
                        nc.scalar.activation(es_[:], sp[:], AF.Exp, scale=HD ** -0.5)
                        if _masked(s, j):
                            nc.vector.tensor_mul(es_[:], es_[:], maskt[j][:])
                        if DBG and s == 0 and hp == 0 and j == 0 and hh == 0:
                            nc.sync.dma_start(d["dbg"][10], es_[:])
                        nc.tensor.matmul(
                            avp[hh][:],
                            v_t[j].rearrange("p (h c) -> p h c", c=65)[:, 2 * hp + hh],
                            es_[:], start=(j == 0), stop=(j == NKV[s] - 1))
                # unload raw + reciprocal of denominator (DVE, off critical path)
                # unload raw; move den to partition 0 via DMA so the fast
                # approx reciprocal (only correct at base partition 0) applies
                raws, rhbs = [], []
                for hh in range(2):
                    raw = p_raw.tile([64, 512], BF16, tag="raw", name=f"rw_{s}_{hp}_{hh}")
                    nc.vector.tensor_copy(raw[:], avp[hh][0:64, :])
                    den = p_rec.tile([65, 512], F32, tag="den", name=f"dn_{s}_{hp}_{hh}")
                    nc.vector.tensor_copy(den[64:65, :], avp[hh][64:65, :])
                    dl0 = p_rec.tile([1, 512], F32, tag="dl0", name=f"dl_{s}_{hp}_{hh}")
                    nc.sync.dma_start(dl0[0:1, :], den[64:65, :])
                    rc0 = p_rec.tile([1, 512], F32, tag="rc0", name=f"rc_{s}_{hp}_{hh}")
                    nc.vector.reciprocal_approx_fast(out=rc0[0:1, :], in_=dl0[0:1, :])
                    rhb = p_rhb.tile([1, 512], BF16, tag="rhb", name=f"rh_{s}_{hp}_{hh}")
                    nc.vector.tensor_copy(rhb[0:1, :], rc0[0:1, :])
                    if DBG and s == 0 and hp == 0:
                        nc.sync.dma_start(d["dbg"][11, hh:hh + 1, :], rhb[0:1, :])
                        nc.sync.dma_start(d["dbg"][11, 2 + hh:3 + hh, :], raw[0:1, :])
                    raws.append(raw)
                    rhbs.append(rhb)
                div_in[(s, hp)] = (raws, rhbs)

            def attn_div(s, hp):
                raws, rhbs = div_in.pop((s, hp))
                rbe = ps_div.tile([P, 512], F32, tag="rb", name=f"rbe_{s}_{hp}")
                nc.tensor.matmul(rbe[0:64, :], ones[0:1, 0:64], rhbs[0][0:1, :],
                                 start=True, stop=True)
                rbo = ps_div.tile([P, 512], F32, tag="ap", name=f"rbo_{s}_{hp}")
                nc.tensor.matmul(rbo[0:64, :], ones[0:1, 0:64], rhbs[1][0:1, :],
                                 start=True, stop=True)
                sc1 = p_sc1.tile([64, 512], BF16, tag="sc1", name=f"sm_{s}_{hp}")
                nc.vector.tensor_mul(sc1[:], raws[1][:], rbo[0:64, :])
                nc.tensor.matmul(rbo[64:128, :], ident[:], sc1[:],
                                 start=True, stop=True, skip_group_check=True)
                nc.vector.tensor_mul(att_t[hp][s][0:64, :], raws[0][:], rbe[0:64, :])
                nc.vector.tensor_copy(att_t[hp][s][64:128, :], rbo[64:128, :])

            # ---------------- P2: attention s0 + fillers ----------------
            with tc.tile_pool(name="p_wvs", bufs=1) as p_wvs, \
                 tc.tile_pool(name="p_wsl2", bufs=2) as p_wsl2, \
                 tc.tile_pool(name="p_msk0", bufs=1) as p_msk0:

                for mi in range(8):
                    mt = p_msk0.tile([P, 512], BF16, tag=f"m{mi}", name=f"mask{mi}")
                    nc.sync.dma_start(mt[:], d["masks"][mi])
                    maskt[mi] = mt
                wvs = [[p_wvs.tile([P, 512], BF16, tag=f"wv{di}_{doc}", name=f"wvs_{di}_{doc}")
                        for doc in range(2)] for di in range(DC)]
                for di in range(DC):
                    for doc in range(2):
                        nc.sync.dma_start(wvs[di][doc][:],
                                          d["wv"][di, :, doc * 512:(doc + 1) * 512])

                def v_proj(j):
                    t5, jo = j // 4, (j % 4) * P
                    for doc in range(2):
                        pt = ps_strm.tile([P, 512], F32, tag="strm", name=f"pv_{j}_{doc}")
                        for di in range(DC):
                            nc.tensor.matmul(pt[:], h_t[di][t5][:, jo:jo + P],
                                             wvs[di][doc][:],
                                             start=(di == 0), stop=(di == DC - 1))
                        dst = v_t[j].rearrange("p (h c) -> p h c", c=65)[:, doc * 8:(doc + 1) * 8, 0:64]
                        nc.scalar.copy(dst, pt.rearrange("p (h c) -> p h c", c=64))
                    nc.scalar.activation(
                        v_t[j].rearrange("p (h c) -> p h c", c=65)[:, :, 64:65],
                        ones[:, 0:H].unsqueeze(2), AF.Copy)

                def kq_unit(dc):
                    wsl = p_wsl2.tile([P, DC, P], BF16, tag="wsl2", name=f"wks2_{dc}")
                    nc.sync.dma_start(wsl[:], d["wk"][dc])
                    for t in (2, 3):
                        pt = ps_strm.tile([P, 512], F32, tag="strm", name=f"p2k_{dc}_{t}")
                        for di in range(DC):
                            nc.tensor.matmul(pt[:], wsl[:, di], h_t[di][t][:],
                                             start=(di == 0), stop=(di == DC - 1))
                        nc.scalar.copy(kt_t[dc][t][:], pt[:])

                def q_unit(dc):
                    wsl = p_wsl2.tile([P, DC, P], BF16, tag="wsl2", name=f"wqs2_{dc}")
                    nc.sync.dma_start(wsl[:], d["wq"][dc])
                    pt = ps_strm.tile([P, 512], F32, tag="strm", name=f"p2q_{dc}")
                    for di in range(DC):
                        nc.tensor.matmul(pt[:], wsl[:, di], hq_t[di][1][:],
                                         start=(di == 0), stop=(di == DC - 1))
                    nc.scalar.copy(qt_t[dc][1][:], pt[:])

                v_proj(0)
                v_proj(1)

                def jit_v(j):
                    if j + 2 < 8:
                        v_proj(j + 2)

                fillers2 = [lambda j=j: v_proj(j) for j in range(8, 16)]
                fi2 = 0
                for hp in range(DC):
                    attn_hp(0, hp, jit_hook=(jit_v if hp == 0 else None))
                    if hp > 0:
                        attn_div(0, hp - 1)
                    if hp > 0:
                        for _ in range(2):
                            if fi2 < len(fillers2):
                                fillers2[fi2]()
                                fi2 += 1
                while fi2 < len(fillers2):
                    fillers2[fi2]()
                    fi2 += 1
                attn_div(0, DC - 1)
                for dc in range(DC):
                    kq_unit(dc)
                for dc in range(DC):
                    q_unit(dc)

            if DBG:
                nc.sync.dma_start(d["dbg"][1], h_t[7][3][:])
                nc.sync.dma_start(d["dbg"][6], kt_t[0][3][:])
                nc.sync.dma_start(d["dbg"][7], qt_t[0][1][:])
                nc.sync.dma_start(d["dbg"][8], v_t[0][:, 0:512])
                nc.sync.dma_start(d["dbg"][9], v_t[15][:, 0:512])
                nc.sync.dma_start(d["dbg"][12], att_t[0][0][:])
                nc.sync.dma_start(d["dbg"][16], kt_t[0][2][:])
                nc.sync.dma_start(d["dbg"][17], kt_t[3][3][:])
                nc.sync.dma_start(d["dbg"][18], v_t[8][:, 0:512])
                nc.sync.dma_start(d["dbg"][19], v_t[2][:, 0:512])

            # slot-1 masks reuse the dead qt(slot0) tile slots in p_main
            for i in range(8):
                mt = p_main.tile([P, 512], BF16, tag=f"pc{2 * i}", name=f"mask{8 + i}")
                nc.sync.dma_start(mt[:], d["masks"][8 + i])
                maskt[8 + i] = mt

            # ---------------- P3 + P4 scope ----------------
            with tc.tile_pool(name="p_wo", bufs=1) as p_wo, \
                 tc.tile_pool(name="p_w1s", bufs=2) as p_w1s, \
                 tc.tile_pool(name="p_w2s", bufs=2) as p_w2s, \
                 tc.tile_pool(name="p_xqf", bufs=1) as p_xqf, \
                 tc.tile_pool(name="p_f2", bufs=1) as p_f2, \
                 tc.tile_pool(name="p_px", bufs=1) as p_px, \
                 tc.tile_pool(name="p_rows2", bufs=3) as p_rows2, \
                 tc.tile_pool(name="p_tmp2", bufs=2) as p_tmp2, \
                 tc.tile_pool(name="p_sq2", bufs=1) as p_sq2:

                x2_t = {}
                for s in range(S):
                    for dc in range(DC):
                        x2_t[(dc, s)] = p_px.tile([P, 512], F32, tag=f"px{dc}",
                                                  name=f"x2_{dc}_{s}")
                xb2_t, h2_t = {}, {}
                for dc in range(DC):
                    xb2_t[(dc, 0)] = p_main.tile([P, 512], BF16, tag=f"pa{32 + dc * S}",
                                                 name=f"xb2_{dc}_0")
                    h2_t[(dc, 0)] = p_main.tile([P, 512], BF16, tag=f"pa{32 + dc * S + 1}",
                                                name=f"h2_{dc}_0")
                    xb2_t[(dc, 1)] = p_main.tile([P, 512], BF16, tag=f"pb{dc * TCKV}",
                                                 name=f"xb2_{dc}_1")
                    h2_t[(dc, 1)] = p_main.tile([P, 512], BF16, tag=f"pb{dc * TCKV + 1}",
                                                name=f"h2_{dc}_1")
                ff1_t = [p_main.tile([P, 512], BF16, tag=f"pa{fc}", name=f"ff1_{fc}")
                         for fc in range(FC)]

                def o_proj(s, dc):
                    wosl = p_wo.tile([P, DC, P], BF16, tag="wo", name=f"wos_{dc}_{s}")
                    nc.sync.dma_start(wosl[:], d["wo"][dc])
                    pt = ps_strm.tile([P, 512], F32, tag="strm", name=f"po_{dc}_{s}")
                    for di in range(DC):
                        nc.tensor.matmul(pt[:], wosl[:, di], att_t[di][s][:],
                                         start=(di == 0), stop=(di == DC - 1))
                    x2t = x2_t[(dc, s)]
                    nc.scalar.activation(x2t[:], pt[:], AF.Identity, bias=bo_col(dc))
                    xqf = p_xqf.tile([P, 512], F32, tag="xqf", name=f"xqf_{dc}_{s}")
                    nc.sync.dma_start(xqf[:], d["xqf"][dc, s])
                    nc.vector.tensor_add(x2t[:], x2t[:], xqf[:])
                    nc.vector.tensor_copy(xb2_t[(dc, s)][:], x2t[:])
                    if DBG and s == 0 and dc == 0:
                        nc.sync.dma_start(d["dbg"][13], xb2_t[(dc, s)][:])

                def ln2_stats(s):
                    xbt = [xb2_t[(dc, s)] for dc in range(DC)]
                    st1 = ps_strm.tile([P, 512], F32, tag="strm", name=f"c2s1_{s}")
                    sqs = []
                    for dc in range(DC):
                        sq = p_sq2.tile([P, 512], BF16, tag="sq2", name=f"c2sq_{dc}_{s}")
                        nc.vector.tensor_mul(sq[:], xbt[dc][:], xbt[dc][:])
                        sqs.append(sq)
                    for dc in range(DC):
                        nc.tensor.matmul(st1[0:1, :], ones[:, 0:1], xbt[dc][:],
                                         start=(dc == 0), stop=(dc == DC - 1))
                    st2 = ps_strm.tile([P, 512], F32, tag="strm", name=f"c2s2_{s}")
                    for dc in range(DC):
                        nc.tensor.matmul(st2[0:1, :], ones[:, 0:1], sqs[dc][:],
                                         start=(dc == 0), stop=(dc == DC - 1))
                    mu = p_rows2.tile([1, 512], F32, tag="rf2", name=f"c2mu_{s}")
                    nc.vector.tensor_scalar_mul(mu[:], st1[0:1, :], 1.0 / D)
                    msq = p_rows2.tile([1, 512], F32, tag="rf2", name=f"c2ms_{s}")
                    nc.vector.tensor_scalar_mul(msq[:], st2[0:1, :], 1.0 / D)
                    var = p_rows2.tile([1, 512], F32, tag="rf2", name=f"c2va_{s}")
                    nc.vector.tensor_mul(var[:], mu[:], mu[:])
                    nc.vector.tensor_sub(var[:], msq[:], var[:])
                    sd = p_rows2.tile([1, 512], F32, tag="rf2", name=f"c2sd_{s}")
                    nc.scalar.activation(sd[:], var[:], AF.Sqrt, bias=epst[:])
                    rsig = p_rows2.tile([1, 512], F32, tag="rf2", name=f"c2rs_{s}")
                    nc.vector.reciprocal_approx_fast(out=rsig[:], in_=sd[:])
                    cmu = p_rows2.tile([1, 512], F32, tag="rf2", name=f"c2cm_{s}")
                    nc.vector.tensor_mul(cmu[:], mu[:], rsig[:])
                    ah = p_rows2.tile([1, 512], BF16, tag="rb2", name=f"c2ah_{s}")
                    nc.vector.tensor_copy(ah[:], rsig[:])
                    ch = p_rows2.tile([1, 512], BF16, tag="rb2", name=f"c2ch_{s}")
                    nc.vector.tensor_copy(ch[:], cmu[:])
                    return (ah, ch)

                def ln2_apply(s, rows):
                    ah, ch = rows
                    bcA = ps_strm.tile([P, 512], F32, tag="strm", name=f"c2bA_{s}")
                    nc.tensor.matmul(bcA[:], ones[0:1, :], ah[:], start=True, stop=True)
                    bcC = ps_strm.tile([P, 512], F32, tag="strm", name=f"c2bC_{s}")
                    nc.tensor.matmul(bcC[:], ones[0:1, :], ch[:], start=True, stop=True)
                    for dc in range(DC):
                        tmp = p_tmp2.tile([P, 512], F32, tag="lnt2", name=f"c2lt_{dc}_{s}")
                        nc.vector.tensor_mul(tmp[:], xb2_t[(dc, s)][:], bcA[:])
                        nc.vector.tensor_sub(tmp[:], tmp[:], bcC[:])
                        nc.scalar.activation(h2_t[(dc, s)][:], tmp[:], AF.Identity,
                                             bias=bl2_col(dc), scale=g2_col(dc))
                    if DBG and s == 0:
                        nc.sync.dma_start(d["dbg"][14], h2_t[(0, s)][:])

                def ffn1(s, fc):
                    pt = ps_strm.tile([P, 512], F32, tag="strm", name=f"pf_{fc}_{s}")
                    for half in range(2):
                        w1s = p_w1s.tile([P, DC // 2, P], BF16, tag="w1s",
                                         name=f"w1s_{s}_{fc}_{half}")
                        nc.sync.dma_start(w1s[:], d["w1"][fc, :, half * 4:half * 4 + 4])
                        for hi in range(DC // 2):
                            di = half * 4 + hi
                            nc.tensor.matmul(pt[:], w1s[:, hi], h2_t[(di, s)][:],
                                             start=(di == 0), stop=(di == DC - 1))
                    nc.scalar.activation(ff1_t[fc][:], pt[:], AF.Relu,
                                         bias=b1t[:, fc:fc + 1])
                    if DBG and s == 0 and fc == 0:
                        nc.sync.dma_start(d["dbg"][15], ff1_t[fc][:])

                def ffn2(s, dc):
                    QW = FC // 4
                    pt = ps_strm.tile([P, 512], F32, tag="strm", name=f"pg_{dc}_{s}")
                    for qi in range(4):
                        w2q = p_w2s.tile([P, QW, P], BF16, tag="w2s", name=f"w2q_{s}_{dc}_{qi}")
                        nc.sync.dma_start(w2q[:], d["w2"][dc, :, qi * QW:(qi + 1) * QW])
                        for fi in range(QW):
                            fc = qi * QW + fi
                            nc.tensor.matmul(pt[:], w2q[:, fi], ff1_t[fc][:],
                                             start=(fc == 0), stop=(fc == FC - 1))
                    f2 = p_f2.tile([P, 512], F32, tag="f2", name=f"f2_{dc}_{s}")
                    nc.scalar.activation(f2[:], pt[:], AF.Relu, bias=b2_col(dc))
                    nc.vector.tensor_add(f2[:], f2[:], x2_t[(dc, s)][:])
                    nc.sync.dma_start(d["outT"][dc, s], f2[:])

                # ---------------- P3: attention s1 + s0 tail ----------------
                ln2_rows = {}
                fillers3 = ([lambda dc=dc: o_proj(0, dc) for dc in range(DC)]
                            + [lambda: ln2_rows.__setitem__(0, ln2_stats(0)),
                               lambda: ln2_apply(0, ln2_rows.pop(0))]
                            + [lambda fc=fc: ffn1(0, fc) for fc in range(FC)]
                            + [lambda dc=dc: ffn2(0, dc) for dc in range(4)])
                fi3 = 0
                for hp in range(DC):
                    attn_hp(1, hp)
                    if hp > 0:
                        attn_div(1, hp - 1)
                    for _ in range(6):
                        if fi3 < len(fillers3):
                            fillers3[fi3]()
                            fi3 += 1
                attn_div(1, DC - 1)
                while fi3 < len(fillers3):
                    fillers3[fi3]()
                    fi3 += 1

                # ---------------- P4: s1 tail (ffn2(0) leftovers fill the
                # serial div/O/LN2 prefix) ----------------
                for dc in range(4):
                    o_proj(1, dc)
                    ffn2(0, 4 + dc)
                for dc in range(4, DC):
                    o_proj(1, dc)
                rows1 = ln2_stats(1)
                ln2_apply(1, rows1)
                for fc in range(FC):
                    ffn1(1, fc)
                for dc in range(DC):
                    ffn2(1, dc)


# ============================ host side ============================

def _slab(w, rows_chunks, cols_chunks):
    r, c = w.shape
    return np.ascontiguousarray(
        w.reshape(rows_chunks, r // rows_chunks, cols_chunks, c // cols_chunks)
        .transpose(2, 1, 0, 3)).astype(ml_dtypes.bfloat16)


def _prep_core(inputs, core):
    b, p = core // 2, core % 2
    bf16 = ml_dtypes.bfloat16
    x = np.asarray(inputs["x"], np.float32)[b]
    xT = np.ascontiguousarray(x.T)
    qb = QBLOCKS[p]
    qidx = np.concatenate([np.arange(q_ * 512, q_ * 512 + 512) for q_ in qb])
    xqT = np.ascontiguousarray(xT[:, qidx])

    m = {}
    m["xkv"] = np.ascontiguousarray(
        xT.reshape(DC, P, TCKV, 512).transpose(0, 2, 1, 3)).astype(bf16)
    xq4 = np.ascontiguousarray(xqT.reshape(DC, P, S, 512).transpose(0, 2, 1, 3))
    m["xqb"] = xq4.astype(bf16)
    m["xqf"] = xq4.astype(np.float32)
    m["wq"] = _slab(np.asarray(inputs["Wq"], np.float32), DC, DC)
    m["wk"] = _slab(np.asarray(inputs["Wk"], np.float32), DC, DC)
    m["wo"] = _slab(np.asarray(inputs["Wo"], np.float32), DC, DC)
    m["wv"] = np.ascontiguousarray(
        np.asarray(inputs["Wv"], np.float32).reshape(DC, P, D)).astype(bf16)
    m["w1"] = _slab(np.asarray(inputs["W1"], np.float32), DC, FC)
    m["w2"] = _slab(np.asarray(inputs["W2"], np.float32), FC, DC)

    masks = np.zeros((NMASK, P, 512), np.float32)
    for s in range(S):
        qstart = qb[s] * 512
        for j in (range(8) if s == 0 else range(8, 16)):
            kv = j * P + np.arange(P)[:, None]
            qg = qstart + np.arange(512)[None, :]
            masks[j] = (kv <= qg).astype(np.float32)
    m["masks"] = masks.astype(bf16)

    biasg = np.zeros((P, 6 * DC), np.float32)
    for i, key in enumerate(["bo", "ln1_g", "ln1_b", "ln2_g", "ln2_b", "b2"]):
        biasg[:, i * DC:(i + 1) * DC] = np.asarray(inputs[key], np.float32).reshape(DC, P).T
    m["biasg"] = np.ascontiguousarray(biasg)
    m["b1c"] = np.ascontiguousarray(
        np.asarray(inputs["b1"], np.float32).reshape(FC, P).T)
    m["onesc"] = np.ones((P, P), bf16)
    m["ident"] = np.eye(64, dtype=np.float32).astype(bf16)
    m["epsv"] = np.full((1, 1), EPS, np.float32)
    return m


def kernel(**inputs):
    if "nc" not in _built:
        _built["nc"] = build_nc()
    nc = _built["nc"]
    in_maps = [_prep_core(inputs, c) for c in range(8)]
    res = run_bass_kernel_spmd(nc, in_maps, core_ids=list(range(8)))
    out = np.zeros((B, T, D), np.float32)
    for c in range(8):
        b, p = c // 2, c % 2
        o = np.asarray(res.results[c]["outT"])
        for s in range(S):
            qb = QBLOCKS[p][s]
            blk = o[:, s].reshape(D, 512)
            out[b, qb * 512:(qb + 1) * 512, :] = blk.T
    return out.astype(np.float32)
